# revision 1
# baseline (speedup 1.0000x reference)
"""Trainium2 Bass kernel for nn_Attention_32091995636193.

Dense transformer attention block:
  qkv = x @ qkv_w.T ; per-head LN(q), LN(k) over head_dim ; k centered over
  seq ; softmax(q*scale @ k^T) @ v ; out @ proj_w.T + proj_b.

Sharding over 8 NeuronCores: data parallel on batch (B=2) x tensor parallel
on heads (16 heads -> 4 per core). Core c handles batch c//4, heads
4*(c%4) .. 4*(c%4)+3. Each core computes its partial projection output
[N, C]; the host sums the 4 partials per batch and adds proj_b.

Per-core device program (all fp32 data, fp32r matmul views):
  1. qT/kT/v from xT and weight slices (natural [n,d] layout for q,k,v)
  2. LayerNorm stats/apply in natural layout (free-axis reductions,
     per-partition scalars), then PE-transpose q,k -> [d, n] layout
  3. k centering over sequence (free-axis in kT layout)
  4. Per head: scores^T = kT.T@qT tiles -> exp (ACT, scale folded) ->
     U = [ones|pad|v].T @ exp accumulated in PSUM; row 0 of U is the
     softmax denominator; rows 32..96 the unnormalized output
  5. normalize via reciprocal + gpsimd partition_broadcast + DVE multiply
  6. projection partial, streamed out per 128-row tile
"""

import os
import sys

for _p in ("/opt/trn_rl_repo",):
    if _p not in sys.path and os.path.isdir(_p):
        sys.path.append(_p)

import numpy as np

# ---------------------------------------------------------------------------
# BIR legalizer: the pinned walrus build supports at most ONE sync wait per
# instruction, but Tile's scheduler attaches several. Split extra waits onto
# NoOp instructions inserted immediately before (same engine => same NX
# order => identical semantics).
# ---------------------------------------------------------------------------
import orjson


def _legalize_bir_json_bytes(raw: bytes) -> bytes:
    j = orjson.loads(raw)
    counter = 0
    for f in j.get("functions", []):
        for blk in f.get("blocks", []):
            insts = blk.get("instructions")
            if not insts:
                continue
            out = []
            for ins in insts:
                si = ins.get("sync_info")
                waits = si.get("on_wait") if si else None
                if waits and len(waits) > 1:
                    engine = ins.get("engine")
                    for w in waits[:-1]:
                        counter += 1
                        nop = {
                            "name": f"WSPLIT-{counter}",
                            "opcode": "NoOp",
                            "engine": engine,
                            "ins": [],
                            "outs": [],
                            "sync_info": {"on_wait": [w], "on_update": []},
                        }
                        if "debug" in ins:
                            nop["debug"] = ins["debug"]
                        out.append(nop)
                    si["on_wait"] = [waits[-1]]
                out.append(ins)
            blk["instructions"] = out
    return orjson.dumps(j)


_PATCHED = False


def _install_patch():
    global _PATCHED
    if _PATCHED:
        return
    import concourse.bass as bass

    orig = bass.Bass.to_json_bytes

    def patched(self):
        return _legalize_bir_json_bytes(orig(self))

    bass.Bass.to_json_bytes = patched
    _PATCHED = True


# ---------------------------------------------------------------------------
# Problem constants (hardcoded per the harness contract)
# ---------------------------------------------------------------------------
B = 2
N = 2048
C = 1024
H = 16
D = 64
SCALE = D ** -0.5
EPS = 1e-5
NCORES = 8
HPC = H // 4          # heads per core = 4
DPC = HPC * D         # channels per core = 256
NT = N // 128         # 16 n-tiles
KT = C // 128         # 8 contraction tiles

_nc_cache = {}


def _build_program(ln_trivial: bool):
    import concourse.bass as bass
    import concourse.mybir as mybir
    import concourse.tile as tile
    from concourse.masks import make_identity
    from concourse import library_config

    fr = mybir.dt.float32r
    f32 = mybir.dt.float32
    bf = mybir.dt.bfloat16
    AX = mybir.AxisListType
    OP = mybir.AluOpType
    ACTF = mybir.ActivationFunctionType

    nc = bass.Bass()
    xt = nc.declare_dram_parameter("xt", [C, N], bf, isOutput=False)
    wq = nc.declare_dram_parameter("wq", [C, DPC], bf, isOutput=False)
    wk = nc.declare_dram_parameter("wk", [C, DPC], bf, isOutput=False)
    wv = nc.declare_dram_parameter("wv", [C, DPC], bf, isOutput=False)
    wp = nc.declare_dram_parameter("wp", [DPC, C], bf, isOutput=False)
    ident_in = nc.declare_dram_parameter("ident", [128, 128], bf, isOutput=False)
    ones64 = nc.declare_dram_parameter("ones64", [1, D], fr, isOutput=False)
    vones = nc.declare_dram_parameter("vones", [128, D], bf, isOutput=False)
    if not ln_trivial:
        gqb = nc.declare_dram_parameter("gqb", [128, 4, D], f32, isOutput=False)
        bqb = nc.declare_dram_parameter("bqb", [128, 4, D], f32, isOutput=False)
        gkb = nc.declare_dram_parameter("gkb", [128, 4, D], f32, isOutput=False)
        bkb = nc.declare_dram_parameter("bkb", [128, 4, D], f32, isOutput=False)
    out = nc.declare_dram_parameter("out", [N, C], f32, isOutput=True)

    with tile.TileContext(nc) as tc:
        with tc.tile_pool(name="const", bufs=1) as cpool, \
             tc.tile_pool(name="persist", bufs=1) as bpool:

            ident = cpool.tile([128, 128], bf)
            nc.scalar.dma_start(ident[:], ident_in[:])

            ones_t = cpool.tile([1, D], fr)
            nc.scalar.dma_start(ones_t[:], ones64[:])
            eps_t = cpool.tile([128, 1], f32)
            nc.vector.memset(eps_t[:], EPS)
            eps64_t = cpool.tile([128, 1], f32)
            nc.vector.memset(eps64_t[:], D * EPS)

            # ---- persistent tensors (live into attention/proj) --------
            wp_s = bpool.tile([128, 2, C], bf, name="wp_s")
            v5 = bpool.tile([128, NT * HPC, 65], bf, name="v5")
            nc.scalar.dma_start(v5[:, :, 64:65], vones.rearrange("p (d o) -> p d o", o=1))
            qT = [bpool.tile([128, N], bf, name=f"qT{p}") for p in range(2)]
            kT = [bpool.tile([128, N], bf, name=f"kT{p}") for p in range(2)]
            outT = [bpool.tile([128, N], bf, name=f"outT{p}") for p in range(2)]

            with tc.tile_pool(name="ph13", bufs=1) as wpool, \
                 tc.tile_pool(name="qkv_ps", bufs=2, space="PSUM") as qps, \
                 tc.tile_pool(name="sq_pool", bufs=2) as sqpool, \
                 tc.tile_pool(name="stat_tmp", bufs=2) as stp, \
                 tc.tile_pool(name="tp_ps", bufs=2, space="PSUM") as tps:
                # ---- load inputs/weights for phases 1-3 ---------------
                wq_s = wpool.tile([128, KT, DPC], bf, name="wq_s")
                nc.scalar.dma_start(wq_s[:], wq.rearrange("(k p) d -> p k d", p=128))
                wk_s = wpool.tile([128, KT, DPC], bf, name="wk_s")
                nc.scalar.dma_start(wk_s[:], wk.rearrange("(k p) d -> p k d", p=128))
                wv_s = wpool.tile([128, KT, DPC], bf, name="wv_s")
                nc.scalar.dma_start(wv_s[:], wv.rearrange("(k p) d -> p k d", p=128))
                xt_s = wpool.tile([128, KT, N], bf, name="xt_s")
                xt_r = xt.rearrange("(k p) n -> p k n", p=128)
                # per-k-tile DMAs so the qkv accumulation (kc-ordered) can
                # start as soon as k-tile 0 lands instead of after all 4MB
                for kc in range(KT):
                    eng = nc.sync if kc % 2 == 0 else nc.scalar
                    eng.dma_start(xt_s[:, kc:kc + 1], xt_r[:, kc:kc + 1])
                # warm the PE clock right before qkv: gate on the xt tile so
                # the burst doesn't fire early and decay while DMAs stream
                wut = qps.tile([128, DPC], f32, tag="pq")
                for i in range(10):
                    nc.tensor.matmul(wut[:], xt_s[:, 0, 0:128],
                                     xt_s[:, 0, 0:DPC], start=True, stop=True)
                if not ln_trivial:
                    gq_s = wpool.tile([128, 4, D], f32, name="gq_s")
                    nc.sync.dma_start(gq_s[:], gqb[:])
                    bq_s = wpool.tile([128, 4, D], f32, name="bq_s")
                    nc.sync.dma_start(bq_s[:], bqb[:])
                    gk_s = wpool.tile([128, 4, D], f32, name="gk_s")
                    nc.sync.dma_start(gk_s[:], gkb[:])
                    bk_s = wpool.tile([128, 4, D], f32, name="bk_s")
                    nc.sync.dma_start(bk_s[:], bkb[:])

                # ---- phases 1-3, half-batched: qkv matmuls + LN stats,
                # stats finalize, apply + transpose. Processing 8-tile halves
                # lets the DVE/ACT LN work of half 0 overlap the PE qkv
                # matmuls of half 1.
                q_nat = wpool.tile([128, NT, 4, D], bf, name="q_nat")
                k_nat = wpool.tile([128, NT, 4, D], bf, name="k_nat")

                s1q = wpool.tile([128, NT, 4], f32, name="s1q")
                s2q = wpool.tile([128, NT, 4], f32, name="s2q")
                s1k = wpool.tile([128, NT, 4], f32, name="s1k")
                s2k = wpool.tile([128, NT, 4], f32, name="s2k")
                mu_q = bpool.tile([128, NT, 4], f32, name="mu_q")
                rs_q = bpool.tile([128, NT, 4], f32, name="rs_q")
                mu_k = bpool.tile([128, NT, 4], f32, name="mu_k")
                rs_k = bpool.tile([128, NT, 4], f32, name="rs_k")

                for half in range(2):
                    h8 = slice(half * 8, half * 8 + 8)
                    for t in range(half * 8, half * 8 + 8):
                        ts_ = slice(t * 128, (t + 1) * 128)
                        pq = qps.tile([128, DPC], f32, tag="pq")
                        pk = qps.tile([128, DPC], f32, tag="pk")
                        pv = qps.tile([128, DPC], f32, tag="pv")
                        for kc in range(KT):
                            st = kc == 0
                            sp = kc == KT - 1
                            lhs = xt_s[:, kc, ts_]
                            nc.tensor.matmul(pk[:], lhs, wk_s[:, kc, :],
                                             start=st, stop=sp)
                            nc.tensor.matmul(pq[:], lhs, wq_s[:, kc, :],
                                             start=st, stop=sp)
                            nc.tensor.matmul(pv[:], lhs, wv_s[:, kc, :],
                                             start=st, stop=sp)
                        # stats straight from PSUM; squares on the (idle) ACT
                        for (pp, s1, s2, natd) in ((pk, s1k, s2k, k_nat),
                                                   (pq, s1q, s2q, q_nat)):
                            pg = pp[:].rearrange("p (g d) -> p g d", g=4)
                            nc.vector.tensor_reduce(s1[:, t], pg, AX.X, OP.add)
                            sq = sqpool.tile([128, 4, D], f32, tag="sq")
                            nc.scalar.square(sq[:], pg)
                            nc.vector.tensor_reduce(s2[:, t], sq[:], AX.X, OP.add)
                            nc.scalar.copy(natd[:, t], pg)
                        nc.scalar.copy(v5[:, t * HPC:(t + 1) * HPC, 0:64],
                                       pv[:].rearrange("p (g d) -> p g d", g=4))

                    # stats finalize for this half
                    # mu = s1/64 ; var = s2/64 - mu^2 ; rstd = 1/sqrt(var+eps)
                    # Trivial-LN k path: rs_k holds SCALE*rstd (the LN mean
                    # subtraction cancels against zero-mean q-hat in the
                    # scores, and rstd*SCALE is applied as the per-partition
                    # exp scale), so k needs no apply at all.
                    for (s1, s2, mu, rs, kfold) in (
                            (s1k, s2k, mu_k, rs_k, ln_trivial),
                            (s1q, s2q, mu_q, rs_q, False)):
                        nc.vector.tensor_scalar(mu[:, h8], s1[:, h8], 1.0 / D,
                                                None, OP.mult)
                        u = stp.tile([128, 8, 4], f32, tag="u")
                        nc.vector.scalar_tensor_tensor(u[:], s1[:, h8], 1.0 / D,
                                                       s1[:, h8], OP.mult, OP.mult)
                        u2 = stp.tile([128, 8, 4], f32, tag="u2")
                        nc.vector.scalar_tensor_tensor(u2[:], u[:], -1.0,
                                                       s2[:, h8], OP.mult, OP.add)
                        if kfold:
                            # rs = 1/sqrt(64*var + 64*eps) = SCALE/sqrt(var+eps)
                            nc.scalar.activation(u[:], u2[:], ACTF.Sqrt,
                                                 bias=eps64_t[:], scale=1.0)
                        else:
                            nc.scalar.activation(u[:], u2[:], ACTF.Sqrt,
                                                 bias=eps_t[:], scale=1.0 / D)
                        nc.vector.reciprocal(rs[:, h8], u[:])

                    # apply for this half (overlaps next half's qkv on PE)
                    apply_list = ([(q_nat, mu_q, rs_q)] if ln_trivial else
                                  [(k_nat, mu_k, rs_k), (q_nat, mu_q, rs_q)])
                    for (nat, mu, rs) in apply_list:
                        for t in range(half * 8, half * 8 + 8):
                            for g in range(4):
                                nc.vector.tensor_scalar(
                                    nat[:, t, g], nat[:, t, g],
                                    mu[:, t, g:g + 1], rs[:, t, g:g + 1],
                                    OP.subtract, OP.mult)
                            if not ln_trivial:
                                gb = gq_s if nat is q_nat else gk_s
                                bb = bq_s if nat is q_nat else bk_s
                                nc.vector.tensor_mul(nat[:, t], nat[:, t], gb[:])
                                nc.vector.tensor_add(nat[:, t], nat[:, t], bb[:])

                    # transposes for this half: k first (no apply dep on the
                    # trivial path, only the ACT psum->k_nat copies), then q
                    # (its apply hides under this half's k transposes / the
                    # next half's qkv matmuls)
                    for (nat, dstT) in ((k_nat, kT), (q_nat, qT)):
                        for s in range(2):
                            for tq in range(2 * half, 2 * half + 2):
                                ptp = tps.tile([128, 4, 128], bf, tag="ptp")
                                for i in range(4):
                                    t = 4 * tq + i
                                    nc.tensor.transpose(
                                        ptp[:, i],
                                        nat[:, t, 2 * s:2 * s + 2, :],
                                        ident[:])
                                if tq % 2 == 0:
                                    nc.vector.tensor_copy(
                                        dstT[s][:, tq * 512:(tq + 1) * 512], ptp[:])
                                else:
                                    nc.scalar.copy(
                                        dstT[s][:, tq * 512:(tq + 1) * 512], ptp[:])

                if not ln_trivial:
                    # center k over sequence (softmax-invariant, kept only
                    # for the general gamma/beta path)
                    with tc.tile_pool(name="ctr", bufs=1) as ctr:
                        for p in range(2):
                            rsum = ctr.tile([128, 1], f32, tag="rsum")
                            nc.vector.tensor_reduce(rsum[:], kT[p][:], AX.X, OP.add)
                            mean = ctr.tile([128, 1], f32, tag="mean")
                            nc.vector.tensor_scalar(mean[:], rsum[:], 1.0 / N,
                                                    None, OP.mult)
                            nc.vector.tensor_scalar(kT[p][:], kT[p][:], mean[:],
                                                    None, OP.subtract)

            # ---- phase 5: attention per head --------------------------
            # Normalization is deferred: during the head loop only raw U
            # and the denominator row are evacuated, keeping PE dense (no
            # DVE-gated PE work between heads -> no HAM re-throttle).
            with tc.tile_pool(name="exp_pool", bufs=4) as epool, \
                 tc.tile_pool(name="nrm_pool", bufs=1) as npool:
                den_all = npool.tile([1, HPC, 2, 1024], f32, name="den_all")
                denr = npool.tile([1, HPC, 2, 1024], fr, name="denr")
                aps_cm = tc.tile_pool(name="att_ps", bufs=1, space="PSUM")
                aps = aps_cm.__enter__()
                # HAM warm-up: half-array matmuls (K=64 scores / M=65 attnv)
                # never un-throttle the PE clock from cold; a short burst of
                # full-array matmuls brings it to 2.4 GHz before the head loop.
                wps = aps.tile([128, 2, 512], f32, tag="ps", bufs=3)
                for i in range(10):
                    nc.tensor.matmul(wps[:, i % 2], kT[0][:, 0:128],
                                     kT[0][:, 0:512], start=True, stop=True)
                # flat chunk sequence with a lag-2 pipeline ACROSS chunk
                # boundaries: the next chunk's scores+exp are emitted before
                # the previous chunk's attnv tail so ACT never drains at
                # (head, nh) transitions. U stays single-buffered (the next
                # chunk's attnv naturally waits for the previous evac).
                chunks = [(h, nh) for h in range(HPC) for nh in range(2)]
                seq = [(ci, mt) for ci in range(len(chunks)) for mt in range(NT)]
                Us = {}
                exs = {}

                def emit_attnv(ci, mt):
                    h, nh = chunks[ci]
                    exv = exs.pop((ci, mt))
                    for j in range(2):
                        nc.tensor.matmul(Us[ci][:, j * 512:(j + 1) * 512],
                                         v5[:, mt * HPC + h, :],
                                         exv[:, j * 512:(j + 1) * 512],
                                         start=(mt == 0), stop=(mt == NT - 1))
                    if mt == NT - 1:
                        p = h // 2
                        off = 64 * (h % 2)
                        nc.vector.tensor_copy(
                            outT[p][off:off + 64, nh * 1024:(nh + 1) * 1024],
                            Us[ci][0:64, :])
                        nc.vector.tensor_copy(den_all[:, h, nh], Us[ci][64:65, :])
                        del Us[ci]

                for i, (ci, mt) in enumerate(seq):
                    h, nh = chunks[ci]
                    if mt == 0:
                        Us[ci] = aps.tile([65, 1024], f32, tag="U", bufs=1,
                                          name=f"U{ci}")
                    p = h // 2
                    off = 64 * (h % 2)
                    ms = slice(mt * 128, (mt + 1) * 128)
                    ps = aps.tile([128, 2, 512], f32, tag="ps", bufs=3)
                    for j in range(2):
                        ns = slice(nh * 1024 + j * 512,
                                   nh * 1024 + (j + 1) * 512)
                        nc.tensor.matmul(ps[:, j], kT[p][off:off + 64, ms],
                                         qT[p][off:off + 64, ns],
                                         start=True, stop=True)
                    ex = epool.tile([128, 1024], bf, tag="ex", bufs=4)
                    esc = rs_k[:, mt, h:h + 1] if ln_trivial else SCALE
                    nc.scalar.activation(ex[:], ps[:].rearrange("p a b -> p (a b)"),
                                         ACTF.Exp, scale=esc)
                    exs[(ci, mt)] = ex
                    if i >= 2:
                        emit_attnv(*seq[i - 2])
                for i in (len(seq) - 2, len(seq) - 1):
                    emit_attnv(*seq[i])

                aps_cm.__exit__(None, None, None)
                # batched reciprocal of all denominators (two head batches):
                # DMA-reshape to 128 partitions so the iterative divide runs
                # on few elems/lane instead of thousands on one lane.
                for bat in range(2):
                    hb = slice(2 * bat, 2 * bat + 2)
                    den128 = npool.tile([128, 32], f32, tag="den128", bufs=2)
                    nc.sync.dma_start(
                        den128[:], den_all[:, hb].rearrange("o h x f -> o (h x f)"))
                    der128 = npool.tile([128, 32], fr, tag="der128", bufs=2)
                    with nc.allow_low_precision("softmax recip"):
                        nc.vector.reciprocal(der128[:], den128[:])
                    nc.sync.dma_start(
                        denr[:, hb].rearrange("o h x f -> o (h x f)"), der128[:])
                nps_cm = tc.tile_pool(name="nrm_ps", bufs=1, space="PSUM")
                nps = nps_cm.__enter__()
                # proj warm-up while normalize finishes
                wpt = nps.tile([128, 512], f32, tag="rbp", bufs=2)
                for i in range(10):
                    nc.tensor.matmul(wpt[:], ident[:], qT[0][:, 0:512],
                                     start=True, stop=True)
                for nh in range(2):
                    for h in range(HPC):
                        p = h // 2
                        off = 64 * (h % 2)
                        rbp = nps.tile([64, 1024], f32, tag="rbp", bufs=2)
                        for j in range(2):
                            nc.tensor.matmul(rbp[:, j * 512:(j + 1) * 512],
                                             ones_t[:],
                                             denr[:, h, nh, j * 512:(j + 1) * 512],
                                             start=True, stop=True)
                        sl = outT[p][off:off + 64, nh * 1024:(nh + 1) * 1024]
                        nc.vector.tensor_mul(sl, sl, rbp[:])
                nps_cm.__exit__(None, None, None)

            # ---- phase 6: projection partial --------------------------
            nc.sync.dma_start(wp_s[:], wp.rearrange("(k p) n -> p k n", p=128))
            with tc.tile_pool(name="prj_ps", bufs=3, space="PSUM") as pps, \
                 tc.tile_pool(name="fin", bufs=4) as fpool:
                for t in range(NT):
                    ts_ = slice(t * 128, (t + 1) * 128)
                    po = pps.tile([128, 2, 512], f32, tag="po")
                    for p in range(2):
                        for j in range(2):
                            nc.tensor.matmul(po[:, j],
                                             outT[p][:, ts_],
                                             wp_s[:, p, j * 512:(j + 1) * 512],
                                             start=(p == 0), stop=(p == 1))
                    fin = fpool.tile([128, 1024], f32, tag="fin")
                    if t % 2 == 0:
                        nc.vector.tensor_copy(fin[:], po[:].rearrange("p a b -> p (a b)"))
                    else:
                        nc.scalar.copy(fin[:], po[:].rearrange("p a b -> p (a b)"))
                    if t % 2 == 0:
                        nc.sync.dma_start(out[ts_, :], fin[:])
                    else:
                        nc.scalar.dma_start(out[ts_, :], fin[:])

    return nc


def _get_program(ln_trivial: bool):
    key = ln_trivial
    if key not in _nc_cache:
        _install_patch()
        _nc_cache[key] = _build_program(ln_trivial)
    return _nc_cache[key]


def _bf16():
    import ml_dtypes
    return ml_dtypes.bfloat16


def _prep_core_inputs(c, x, qkv_w, q_norm_w, q_norm_b, k_norm_w, k_norm_b,
                      proj_w, ln_trivial):
    b = c // 4
    g = c % 4
    rows = slice(g * DPC, (g + 1) * DPC)
    b16 = _bf16()
    xt = np.ascontiguousarray(x[b].T).astype(b16)           # [C, N]
    wq = np.ascontiguousarray(qkv_w[rows, :].T).astype(b16)  # [C, DPC]
    wk = np.ascontiguousarray(qkv_w[C:2 * C, :][rows, :].T).astype(b16)
    wv = np.ascontiguousarray(qkv_w[2 * C:3 * C, :][rows, :].T).astype(b16)
    wp = np.ascontiguousarray(proj_w[:, rows].T).astype(b16)  # [DPC, C]
    m = {"xt": xt, "wq": wq, "wk": wk, "wv": wv, "wp": wp,
         "ident": np.eye(128, dtype=_bf16()),
         "ones64": np.ones((1, D), dtype=np.float32),
         "vones": np.ones((128, D), dtype=_bf16())}
    if not ln_trivial:
        for nm, arr in (("gqb", q_norm_w), ("bqb", q_norm_b),
                        ("gkb", k_norm_w), ("bkb", k_norm_b)):
            t = np.broadcast_to(arr.astype(np.float32), (128, 4, D))
            m[nm] = np.ascontiguousarray(t)
    return m


def kernel(x, qkv_w, q_norm_w, q_norm_b, k_norm_w, k_norm_b, proj_w, proj_b,
           _trace=False):
    from concourse.bass_utils import run_bass_kernel_spmd

    x = np.asarray(x, dtype=np.float32)
    qkv_w = np.asarray(qkv_w, dtype=np.float32)
    q_norm_w = np.asarray(q_norm_w, dtype=np.float32)
    q_norm_b = np.asarray(q_norm_b, dtype=np.float32)
    k_norm_w = np.asarray(k_norm_w, dtype=np.float32)
    k_norm_b = np.asarray(k_norm_b, dtype=np.float32)
    proj_w = np.asarray(proj_w, dtype=np.float32)
    proj_b = np.asarray(proj_b, dtype=np.float32)

    ln_trivial = (np.all(q_norm_w == 1.0) and np.all(q_norm_b == 0.0)
                  and np.all(k_norm_w == 1.0) and np.all(k_norm_b == 0.0))

    nc = _get_program(ln_trivial)
    in_maps = [
        _prep_core_inputs(c, x, qkv_w, q_norm_w, q_norm_b, k_norm_w,
                          k_norm_b, proj_w, ln_trivial)
        for c in range(NCORES)
    ]
    res = run_bass_kernel_spmd(nc, in_maps, list(range(NCORES)),
                               trace=_trace)
    outs = [res.results[c]["out"] for c in range(NCORES)]
    full = np.empty((B, N, C), dtype=np.float32)
    for b in range(B):
        acc = outs[4 * b].astype(np.float32)
        for g in range(1, 4):
            acc = acc + outs[4 * b + g]
        full[b] = acc + proj_b[None, :]
    if _trace:
        return full, res
    return full



# revision 14
# speedup vs baseline: 1.0395x; 1.0395x over previous
"""Trainium2 Bass kernel for nn_Attention_32091995636193.

Dense transformer attention block:
  qkv = x @ qkv_w.T ; per-head LN(q), LN(k) over head_dim ; k centered over
  seq ; softmax(q*scale @ k^T) @ v ; out @ proj_w.T + proj_b.

Sharding over 8 NeuronCores: data parallel on batch (B=2) x tensor parallel
on heads (16 heads -> 4 per core). Core c handles batch c//4, heads
4*(c%4) .. 4*(c%4)+3. Each core computes its partial projection output
[N, C]; the host sums the 4 partials per batch and adds proj_b.

Per-core device program (bf16 data, fp32 psum):
  1. qT/kT/v from xT and weight slices (natural [n,d] layout)
  2. LayerNorm stats from PSUM, finalize, apply. Trivial-gamma path folds
     SCALE*rstd_k into a k-scale pass (LN mean of k cancels against
     zero-mean q-hat inside the softmax; see notes below), so exp runs
     with a uniform scale of 1.0.
  3. q/k -> [d, n] layout via DMA-xbar transposes (no PE time)
  4. Attention per head-PAIR: the two heads of a pair live on PE row
     strips 0-63 / 64-127, so their K=64 score matmuls execute
     concurrently (row tiling). Per (pair, n-quarter, mt): one
     [128,2,512] score tile -> one [128,1024] EXP on ACT -> two [65,512]
     attnv matmuls accumulating U (row 64 = ones = softmax denominator).
     ACT does nothing but EXP in this phase - it is the wall.
  5. deferred normalize: reciprocal of denominators + ones-matmul
     broadcast + DVE multiply
  6. projection partial, streamed out per 128-row tile
"""

import os
import sys

for _p in ("/opt/trn_rl_repo",):
    if _p not in sys.path and os.path.isdir(_p):
        sys.path.append(_p)

import numpy as np

# ---------------------------------------------------------------------------
# BIR legalizer: the pinned walrus build supports at most ONE sync wait per
# instruction, but Tile's scheduler attaches several. Split extra waits onto
# NoOp instructions inserted immediately before (same engine => same NX
# order => identical semantics).
# ---------------------------------------------------------------------------
import orjson


def _legalize_bir_json_bytes(raw: bytes) -> bytes:
    j = orjson.loads(raw)
    counter = 0
    for f in j.get("functions", []):
        for blk in f.get("blocks", []):
            insts = blk.get("instructions")
            if not insts:
                continue
            out = []
            for ins in insts:
                si = ins.get("sync_info")
                waits = si.get("on_wait") if si else None
                if waits and len(waits) > 1:
                    engine = ins.get("engine")
                    for w in waits[:-1]:
                        counter += 1
                        nop = {
                            "name": f"WSPLIT-{counter}",
                            "opcode": "NoOp",
                            "engine": engine,
                            "ins": [],
                            "outs": [],
                            "sync_info": {"on_wait": [w], "on_update": []},
                        }
                        if "debug" in ins:
                            nop["debug"] = ins["debug"]
                        out.append(nop)
                    si["on_wait"] = [waits[-1]]
                out.append(ins)
            blk["instructions"] = out
    return orjson.dumps(j)


_PATCHED = False


def _install_patch():
    global _PATCHED
    if _PATCHED:
        return
    import concourse.bass as bass

    orig = bass.Bass.to_json_bytes

    def patched(self):
        return _legalize_bir_json_bytes(orig(self))

    bass.Bass.to_json_bytes = patched
    _PATCHED = True


# ---------------------------------------------------------------------------
# Problem constants (hardcoded per the harness contract)
# ---------------------------------------------------------------------------
B = 2
N = 2048
C = 1024
H = 16
D = 64
SCALE = D ** -0.5
EPS = 1e-5
NCORES = 8
HPC = H // 4          # heads per core = 4
DPC = HPC * D         # channels per core = 256
NT = N // 128         # 16 n-tiles
KT = C // 128         # 8 contraction tiles

_nc_cache = {}


def _build_program(ln_trivial: bool):
    import concourse.bass as bass
    import concourse.mybir as mybir
    import concourse.tile as tile

    fr = mybir.dt.float32r
    f32 = mybir.dt.float32
    bf = mybir.dt.bfloat16
    AX = mybir.AxisListType
    OP = mybir.AluOpType
    ACTF = mybir.ActivationFunctionType

    nc = bass.Bass()
    xt = nc.declare_dram_parameter("xt", [C, N], bf, isOutput=False)
    wq = nc.declare_dram_parameter("wq", [C, DPC], bf, isOutput=False)
    wk = nc.declare_dram_parameter("wk", [C, DPC], bf, isOutput=False)
    wv = nc.declare_dram_parameter("wv", [C, DPC], bf, isOutput=False)
    wp = nc.declare_dram_parameter("wp", [DPC, C], bf, isOutput=False)
    ident_in = nc.declare_dram_parameter("ident", [128, 128], bf, isOutput=False)
    ones64 = nc.declare_dram_parameter("ones64", [1, D], fr, isOutput=False)
    vones = nc.declare_dram_parameter("vones", [128, D], bf, isOutput=False)
    if not ln_trivial:
        gqb = nc.declare_dram_parameter("gqb", [128, 4, D], f32, isOutput=False)
        bqb = nc.declare_dram_parameter("bqb", [128, 4, D], f32, isOutput=False)
        gkb = nc.declare_dram_parameter("gkb", [128, 4, D], f32, isOutput=False)
        bkb = nc.declare_dram_parameter("bkb", [128, 4, D], f32, isOutput=False)
    out = nc.declare_dram_parameter("out", [N, C], f32, isOutput=True)

    with tile.TileContext(nc) as tc:
        with tc.tile_pool(name="const", bufs=1) as cpool, \
             tc.tile_pool(name="persist", bufs=1) as bpool:

            ident = cpool.tile([128, 128], bf)
            nc.scalar.dma_start(ident[:], ident_in[:])
            ones_t = cpool.tile([1, D], fr)
            nc.scalar.dma_start(ones_t[:], ones64[:])
            eps_t = cpool.tile([128, 1], f32)
            nc.vector.memset(eps_t[:], EPS)
            eps64_t = cpool.tile([128, 1], f32)
            nc.vector.memset(eps64_t[:], D * EPS)

            # ---- persistent tensors (live into attention/proj) --------
            wp_s = bpool.tile([128, 2, C], bf, name="wp_s")
            v5 = bpool.tile([128, NT * HPC, 65], bf, name="v5")
            nc.scalar.dma_start(v5[:, :, 64:65], vones.rearrange("p (d o) -> p d o", o=1))
            qT = [bpool.tile([128, N], bf, name=f"qT{p}") for p in range(2)]
            kT = [bpool.tile([128, N], bf, name=f"kT{p}") for p in range(2)]
            outT = [bpool.tile([128, N], bf, name=f"outT{p}") for p in range(2)]

            with tc.tile_pool(name="ph13", bufs=1) as wpool, \
                 tc.tile_pool(name="qkv_ps", bufs=2, space="PSUM") as qps, \
                 tc.tile_pool(name="sq_pool", bufs=2) as sqpool, \
                 tc.tile_pool(name="stat_tmp", bufs=2) as stp, \
                 tc.tile_pool(name="tp_ps", bufs=2, space="PSUM") as tps:
                # ---- load inputs/weights for phases 1-3 ---------------
                wq_s = wpool.tile([128, KT, DPC], bf, name="wq_s")
                nc.scalar.dma_start(wq_s[:], wq.rearrange("(k p) d -> p k d", p=128))
                wk_s = wpool.tile([128, KT, DPC], bf, name="wk_s")
                nc.scalar.dma_start(wk_s[:], wk.rearrange("(k p) d -> p k d", p=128))
                wv_s = wpool.tile([128, KT, DPC], bf, name="wv_s")
                nc.scalar.dma_start(wv_s[:], wv.rearrange("(k p) d -> p k d", p=128))
                xt_s = wpool.tile([128, KT, N], bf, name="xt_s")
                xt_r = xt.rearrange("(k p) n -> p k n", p=128)
                # per-k-tile DMAs so the qkv accumulation (kc-ordered) can
                # start as soon as k-tile 0 lands instead of after all 4MB
                for kc in range(KT):
                    eng = nc.sync if kc % 2 == 0 else nc.scalar
                    eng.dma_start(xt_s[:, kc:kc + 1], xt_r[:, kc:kc + 1])
                # warm the PE clock right before qkv: gate on the xt tile so
                # the burst doesn't fire early and decay while DMAs stream
                wut = qps.tile([128, DPC], f32, tag="pq")
                for i in range(10):
                    nc.tensor.matmul(wut[:], xt_s[:, 0, 0:128],
                                     xt_s[:, 0, 0:DPC], start=True, stop=True)
                if not ln_trivial:
                    gq_s = wpool.tile([128, 4, D], f32, name="gq_s")
                    nc.sync.dma_start(gq_s[:], gqb[:])
                    bq_s = wpool.tile([128, 4, D], f32, name="bq_s")
                    nc.sync.dma_start(bq_s[:], bqb[:])
                    gk_s = wpool.tile([128, 4, D], f32, name="gk_s")
                    nc.sync.dma_start(gk_s[:], gkb[:])
                    bk_s = wpool.tile([128, 4, D], f32, name="bk_s")
                    nc.sync.dma_start(bk_s[:], bkb[:])

                # ---- phases 1-3, half-batched: qkv matmuls + LN stats,
                # stats finalize, apply + transpose. Processing 8-tile halves
                # lets the DVE/ACT LN work of half 0 overlap the PE qkv
                # matmuls of half 1.
                q_nat = wpool.tile([128, NT, 4, D], bf, name="q_nat")
                k_nat = wpool.tile([128, NT, 4, D], bf, name="k_nat")
                q_app = wpool.tile([128, NT, 4, D], bf, name="q_app")
                k_app = wpool.tile([128, NT, 4, D], bf, name="k_app")

                s1q = wpool.tile([128, NT, 4], f32, name="s1q")
                s2q = wpool.tile([128, NT, 4], f32, name="s2q")
                s1k = wpool.tile([128, NT, 4], f32, name="s1k")
                s2k = wpool.tile([128, NT, 4], f32, name="s2k")
                mu_q = bpool.tile([128, NT, 4], f32, name="mu_q")
                rs_q = bpool.tile([128, NT, 4], f32, name="rs_q")
                mu_k = bpool.tile([128, NT, 4], f32, name="mu_k")
                rs_k = bpool.tile([128, NT, 4], f32, name="rs_k")

                for half in range(2):
                    h8 = slice(half * 8, half * 8 + 8)
                    for t in range(half * 8, half * 8 + 8):
                        ts_ = slice(t * 128, (t + 1) * 128)
                        pq = qps.tile([128, DPC], f32, tag="pq")
                        pk = qps.tile([128, DPC], f32, tag="pk")
                        pv = qps.tile([128, DPC], f32, tag="pv")
                        for kc in range(KT):
                            st = kc == 0
                            sp = kc == KT - 1
                            lhs = xt_s[:, kc, ts_]
                            nc.tensor.matmul(pk[:], lhs, wk_s[:, kc, :],
                                             start=st, stop=sp)
                            nc.tensor.matmul(pq[:], lhs, wq_s[:, kc, :],
                                             start=st, stop=sp)
                            nc.tensor.matmul(pv[:], lhs, wv_s[:, kc, :],
                                             start=st, stop=sp)
                        # stats straight from PSUM; squares on the (idle) ACT
                        for (pp, s1, s2, natd) in ((pk, s1k, s2k, k_nat),
                                                   (pq, s1q, s2q, q_nat)):
                            pg = pp[:].rearrange("p (g d) -> p g d", g=4)
                            nc.vector.tensor_reduce(s1[:, t], pg, AX.X, OP.add)
                            sq = sqpool.tile([128, 4, D], f32, tag="sq")
                            nc.scalar.square(sq[:], pg)
                            nc.vector.tensor_reduce(s2[:, t], sq[:], AX.X, OP.add)
                            nc.scalar.copy(natd[:, t], pg)
                        nc.scalar.copy(v5[:, t * HPC:(t + 1) * HPC, 0:64],
                                       pv[:].rearrange("p (g d) -> p g d", g=4))

                    # stats finalize for this half
                    # mu = s1/64 ; var = s2/64 - mu^2 ; rstd = 1/sqrt(var+eps)
                    # Trivial-LN k path: rs_k holds SCALE*rstd; it is applied
                    # multiplicatively to k_nat below (the LN mean
                    # subtraction cancels against zero-mean q-hat in the
                    # scores), giving a uniform exp scale of 1.0.
                    for (s1, s2, mu, rs, kfold) in (
                            (s1k, s2k, mu_k, rs_k, ln_trivial),
                            (s1q, s2q, mu_q, rs_q, False)):
                        nc.vector.tensor_scalar(mu[:, h8], s1[:, h8], 1.0 / D,
                                                None, OP.mult)
                        u = stp.tile([128, 8, 4], f32, tag="u")
                        nc.vector.scalar_tensor_tensor(u[:], s1[:, h8], 1.0 / D,
                                                       s1[:, h8], OP.mult, OP.mult)
                        u2 = stp.tile([128, 8, 4], f32, tag="u2")
                        nc.vector.scalar_tensor_tensor(u2[:], u[:], -1.0,
                                                       s2[:, h8], OP.mult, OP.add)
                        if kfold:
                            # rs = 1/sqrt(64*var + 64*eps) = SCALE/sqrt(var+eps)
                            nc.scalar.activation(u[:], u2[:], ACTF.Sqrt,
                                                 bias=eps64_t[:], scale=1.0)
                        else:
                            nc.scalar.activation(u[:], u2[:], ACTF.Sqrt,
                                                 bias=eps_t[:], scale=1.0 / D)
                        nc.vector.reciprocal(rs[:, h8], u[:])

                    # apply for this half (overlaps next half's qkv on PE).
                    # Trivial path: q gets (x-mu)*rs; k gets x*rs_k with
                    # rs_k = SCALE*rstd (no mean subtraction needed).
                    for t in range(half * 8, half * 8 + 8):
                        for g in range(4):
                            nc.vector.tensor_scalar(
                                q_app[:, t, g], q_nat[:, t, g],
                                mu_q[:, t, g:g + 1], rs_q[:, t, g:g + 1],
                                OP.subtract, OP.mult)
                        if ln_trivial:
                            for g in range(4):
                                nc.vector.tensor_scalar(
                                    k_app[:, t, g], k_nat[:, t, g],
                                    rs_k[:, t, g:g + 1], None, OP.mult)
                        else:
                            for g in range(4):
                                nc.vector.tensor_scalar(
                                    k_app[:, t, g], k_nat[:, t, g],
                                    mu_k[:, t, g:g + 1], rs_k[:, t, g:g + 1],
                                    OP.subtract, OP.mult)
                            nc.vector.tensor_mul(q_app[:, t], q_app[:, t], gq_s[:])
                            nc.vector.tensor_add(q_app[:, t], q_app[:, t], bq_s[:])
                            nc.vector.tensor_mul(k_app[:, t], k_app[:, t], gk_s[:])
                            nc.vector.tensor_add(k_app[:, t], k_app[:, t], bk_s[:])

                    # transposes for this half on the PE (DMA-xbar transpose
                    # has unreliable completion ordering in this stack)
                    for (nat, dstT) in ((k_app, kT), (q_app, qT)):
                        for s in range(2):
                            for tq in range(2 * half, 2 * half + 2):
                                ptp = tps.tile([128, 4, 128], bf, tag="ptp")
                                for i in range(4):
                                    t = 4 * tq + i
                                    nc.tensor.transpose(
                                        ptp[:, i],
                                        nat[:, t, 2 * s:2 * s + 2, :],
                                        ident[:])
                                if tq % 2 == 0:
                                    nc.vector.tensor_copy(
                                        dstT[s][:, tq * 512:(tq + 1) * 512], ptp[:])
                                else:
                                    nc.scalar.copy(
                                        dstT[s][:, tq * 512:(tq + 1) * 512], ptp[:])

                if not ln_trivial:
                    # center k over sequence (softmax-invariant, kept only
                    # for the general gamma/beta path)
                    with tc.tile_pool(name="ctr", bufs=1) as ctr:
                        for p in range(2):
                            rsum = ctr.tile([128, 1], f32, tag="rsum")
                            nc.vector.tensor_reduce(rsum[:], kT[p][:], AX.X, OP.add)
                            mean = ctr.tile([128, 1], f32, tag="mean")
                            nc.vector.tensor_scalar(mean[:], rsum[:], 1.0 / N,
                                                    None, OP.mult)
                            nc.vector.tensor_scalar(kT[p][:], kT[p][:], mean[:],
                                                    None, OP.subtract)

            # ---- phase 5+6: attention per head-pair, with the per-q
            # normalize + projection + output DMA interleaved under the
            # NEXT q-chunk's (ACT-bound) attention sweeps.
            # Heads (2p, 2p+1) live on PE row strips 0-63 / 64-127 of
            # kT[p]/qT[p], so their K=64 score matmuls run CONCURRENTLY
            # (row tiling). One [128,2,512] score tile per (pair, q, mt)
            # feeds a single [128,1024] EXP - ACT does only EXP here.
            esc = 1.0 if ln_trivial else SCALE
            nc.sync.dma_start(wp_s[:], wp.rearrange("(k p) n -> p k n", p=128))
            with tc.tile_pool(name="exp_pool", bufs=3) as epool, \
                 tc.tile_pool(name="nrm_pool", bufs=1) as npool, \
                 tc.tile_pool(name="fin", bufs=4) as fpool, \
                 tc.tile_pool(name="att_ps", bufs=1, space="PSUM") as aps:
                den_all = npool.tile([1, 4, HPC, 512], f32, name="den_all")
                denr = npool.tile([1, 4, HPC, 512], fr, name="denr")
                # warm-up burst before the head loop
                wps = aps.tile([128, 2, 512], f32, tag="ps", bufs=2)
                for i in range(10):
                    nc.tensor.matmul(wps[:, i % 2], kT[0][:, 0:128],
                                     kT[0][:, 0:512], start=True, stop=True)

                def attention_sweep(hA, hB, q):
                    p = hA // 2
                    ns = slice(q * 512, (q + 1) * 512)
                    U = {}
                    for (slot, h) in ((0, hA), (1, hB)):
                        U[slot] = aps.tile([65, 512], f32, tag=f"U{slot}",
                                           bufs=1, name=f"U{h}_{q}")
                    exs = {}

                    def emit_attnv(mt, flush=False):
                        exv = exs.pop(mt)
                        for (slot, h) in ((0, hA), (1, hB)):
                            nc.tensor.matmul(
                                U[slot][:],
                                v5[:, mt * HPC + h, :],
                                exv[:, slot, :],
                                start=(mt == 0), stop=(mt == NT - 1))
                        if flush:
                            for (slot, h) in ((0, hA), (1, hB)):
                                off = 64 * (h % 2)
                                nc.vector.tensor_copy(
                                    outT[p][off:off + 64, ns],
                                    U[slot][0:64, :])
                                nc.vector.tensor_copy(
                                    den_all[:, q, h], U[slot][64:65, :])

                    for mt in range(NT):
                        ms = slice(mt * 128, (mt + 1) * 128)
                        ps = aps.tile([128, 2, 512], f32, tag="ps", bufs=2)
                        for (slot, h) in ((0, hA), (1, hB)):
                            off = 64 * (h % 2)
                            nc.tensor.matmul(ps[:, slot],
                                             kT[p][off:off + 64, ms],
                                             qT[p][off:off + 64, ns],
                                             start=True, stop=True)
                        ex = epool.tile([128, 2, 512], bf, tag="ex", bufs=3)
                        nc.scalar.activation(
                            ex[:].rearrange("p a b -> p (a b)"),
                            ps[:].rearrange("p a b -> p (a b)"),
                            ACTF.Exp, scale=esc)
                        exs[mt] = ex
                        if mt >= 1:
                            emit_attnv(mt - 1)
                    emit_attnv(NT - 1, flush=True)

                def finish_q(q):
                    # reciprocal of this q-chunk's denominators, directly on
                    # partition 0 (slow per-lane, but hidden under the next
                    # q-chunk's ACT-bound attention)
                    ns = slice(q * 512, (q + 1) * 512)
                    with nc.allow_low_precision("softmax recip"):
                        nc.vector.reciprocal(denr[:, q], den_all[:, q])
                    # broadcast 1/den down the partitions via K=1 matmuls
                    # (dst must start at partition 0; the DVE multiply can
                    # read cross-partition, so heads 1/3 read rows 0-63 while
                    # writing outT rows 64-127)
                    for hp in range(2):
                        rbp = aps.tile([128, 2, 512], f32, tag="po", bufs=1)
                        for j in range(2):
                            h = 2 * hp + j
                            nc.tensor.matmul(rbp[0:64, j, :],
                                             ones_t[:], denr[:, q, h],
                                             start=True, stop=True)
                        for j in range(2):
                            h = 2 * hp + j
                            p = h // 2
                            off = 64 * (h % 2)
                            sl = outT[p][off:off + 64, ns]
                            nc.vector.tensor_mul(sl, sl, rbp[0:64, j, :])
                    # projection for the 4 n-tiles of this q-chunk
                    for t in range(4 * q, 4 * q + 4):
                        ts_ = slice(t * 128, (t + 1) * 128)
                        po = aps.tile([128, 2, 512], f32, tag="po", bufs=1)
                        for p in range(2):
                            for j in range(2):
                                nc.tensor.matmul(po[:, j],
                                                 outT[p][:, ts_],
                                                 wp_s[:, p, j * 512:(j + 1) * 512],
                                                 start=(p == 0), stop=(p == 1))
                        fin = fpool.tile([128, 1024], f32, tag="fin")
                        nc.vector.tensor_copy(fin[:],
                                              po[:].rearrange("p a b -> p (a b)"))
                        nc.sync.dma_start(out[ts_, :], fin[:])

                for q in range(4):
                    for (hA, hB) in ((0, 1), (2, 3)):
                        attention_sweep(hA, hB, q)
                    finish_q(q)

    return nc


def _get_program(ln_trivial: bool):
    key = ln_trivial
    if key not in _nc_cache:
        _install_patch()
        _nc_cache[key] = _build_program(ln_trivial)
    return _nc_cache[key]


def _bf16():
    import ml_dtypes
    return ml_dtypes.bfloat16


def _prep_core_inputs(c, x, qkv_w, q_norm_w, q_norm_b, k_norm_w, k_norm_b,
                      proj_w, ln_trivial):
    b = c // 4
    g = c % 4
    rows = slice(g * DPC, (g + 1) * DPC)
    b16 = _bf16()
    xt = np.ascontiguousarray(x[b].T).astype(b16)           # [C, N]
    wq = np.ascontiguousarray(qkv_w[rows, :].T).astype(b16)  # [C, DPC]
    wk = np.ascontiguousarray(qkv_w[C:2 * C, :][rows, :].T).astype(b16)
    wv = np.ascontiguousarray(qkv_w[2 * C:3 * C, :][rows, :].T).astype(b16)
    wp = np.ascontiguousarray(proj_w[:, rows].T).astype(b16)  # [DPC, C]
    m = {"xt": xt, "wq": wq, "wk": wk, "wv": wv, "wp": wp,
         "ident": np.eye(128, dtype=_bf16()),
         "ones64": np.ones((1, D), dtype=np.float32),
         "vones": np.ones((128, D), dtype=_bf16())}
    if not ln_trivial:
        for nm, arr in (("gqb", q_norm_w), ("bqb", q_norm_b),
                        ("gkb", k_norm_w), ("bkb", k_norm_b)):
            t = np.broadcast_to(arr.astype(np.float32), (128, 4, D))
            m[nm] = np.ascontiguousarray(t)
    return m


def kernel(x, qkv_w, q_norm_w, q_norm_b, k_norm_w, k_norm_b, proj_w, proj_b,
           _trace=False):
    from concourse.bass_utils import run_bass_kernel_spmd

    x = np.asarray(x, dtype=np.float32)
    qkv_w = np.asarray(qkv_w, dtype=np.float32)
    q_norm_w = np.asarray(q_norm_w, dtype=np.float32)
    q_norm_b = np.asarray(q_norm_b, dtype=np.float32)
    k_norm_w = np.asarray(k_norm_w, dtype=np.float32)
    k_norm_b = np.asarray(k_norm_b, dtype=np.float32)
    proj_w = np.asarray(proj_w, dtype=np.float32)
    proj_b = np.asarray(proj_b, dtype=np.float32)

    ln_trivial = (np.all(q_norm_w == 1.0) and np.all(q_norm_b == 0.0)
                  and np.all(k_norm_w == 1.0) and np.all(k_norm_b == 0.0))

    nc = _get_program(ln_trivial)
    in_maps = [
        _prep_core_inputs(c, x, qkv_w, q_norm_w, q_norm_b, k_norm_w,
                          k_norm_b, proj_w, ln_trivial)
        for c in range(NCORES)
    ]
    res = run_bass_kernel_spmd(nc, in_maps, list(range(NCORES)),
                               trace=_trace)
    outs = [res.results[c]["out"] for c in range(NCORES)]
    full = np.empty((B, N, C), dtype=np.float32)
    for b in range(B):
        acc = outs[4 * b].astype(np.float32)
        for g in range(1, 4):
            acc = acc + outs[4 * b + g]
        full[b] = acc + proj_b[None, :]
    if _trace:
        return full, res
    return full


# revision 16
# speedup vs baseline: 1.2057x; 1.1599x over previous
"""Trainium2 Bass kernel for nn_Attention_32091995636193.

Dense transformer attention block:
  qkv = x @ qkv_w.T ; per-head LN(q), LN(k) over head_dim ; k centered over
  seq ; softmax(q*scale @ k^T) @ v ; out @ proj_w.T + proj_b.

Sharding over 8 NeuronCores: data parallel on batch (B=2) x tensor parallel
on heads (16 heads -> 4 per core). Core c handles batch c//4, heads
4*(c%4) .. 4*(c%4)+3. Each core computes its partial projection output
[N, C]; the host sums the 4 partials per batch and adds proj_b.

Per-core device program (bf16 data, fp32 psum):
  1. qT/kT/v from xT and weight slices (natural [n,d] layout)
  2. LayerNorm stats from PSUM, finalize, apply. Trivial-gamma path folds
     SCALE*rstd_k into a k-scale pass (LN mean of k cancels against
     zero-mean q-hat inside the softmax; see notes below), so exp runs
     with a uniform scale of 1.0.
  3. q/k -> [d, n] layout via DMA-xbar transposes (no PE time)
  4. Attention per head-PAIR: the two heads of a pair live on PE row
     strips 0-63 / 64-127, so their K=64 score matmuls execute
     concurrently (row tiling). Per (pair, n-quarter, mt): one
     [128,2,512] score tile -> one [128,1024] EXP on ACT -> two [65,512]
     attnv matmuls accumulating U (row 64 = ones = softmax denominator).
     ACT does nothing but EXP in this phase - it is the wall.
  5. deferred normalize: reciprocal of denominators + ones-matmul
     broadcast + DVE multiply
  6. projection partial, streamed out per 128-row tile
"""

import os
import sys

for _p in ("/opt/trn_rl_repo",):
    if _p not in sys.path and os.path.isdir(_p):
        sys.path.append(_p)

import numpy as np

# ---------------------------------------------------------------------------
# BIR legalizer: the pinned walrus build supports at most ONE sync wait per
# instruction, but Tile's scheduler attaches several. Split extra waits onto
# NoOp instructions inserted immediately before (same engine => same NX
# order => identical semantics).
# ---------------------------------------------------------------------------
import orjson


def _legalize_bir_json_bytes(raw: bytes) -> bytes:
    j = orjson.loads(raw)
    counter = 0
    for f in j.get("functions", []):
        for blk in f.get("blocks", []):
            insts = blk.get("instructions")
            if not insts:
                continue
            out = []
            for ins in insts:
                si = ins.get("sync_info")
                waits = si.get("on_wait") if si else None
                if waits and len(waits) > 1:
                    engine = ins.get("engine")
                    for w in waits[:-1]:
                        counter += 1
                        nop = {
                            "name": f"WSPLIT-{counter}",
                            "opcode": "NoOp",
                            "engine": engine,
                            "ins": [],
                            "outs": [],
                            "sync_info": {"on_wait": [w], "on_update": []},
                        }
                        if "debug" in ins:
                            nop["debug"] = ins["debug"]
                        out.append(nop)
                    si["on_wait"] = [waits[-1]]
                out.append(ins)
            blk["instructions"] = out
    return orjson.dumps(j)


_PATCHED = False


def _install_patch():
    global _PATCHED
    if _PATCHED:
        return
    import concourse.bass as bass

    orig = bass.Bass.to_json_bytes

    def patched(self):
        return _legalize_bir_json_bytes(orig(self))

    bass.Bass.to_json_bytes = patched
    _PATCHED = True


# ---------------------------------------------------------------------------
# Problem constants (hardcoded per the harness contract)
# ---------------------------------------------------------------------------
B = 2
N = 2048
C = 1024
H = 16
D = 64
SCALE = D ** -0.5
EPS = 1e-5
NCORES = 8
HPC = H // 4          # heads per core = 4
DPC = HPC * D         # channels per core = 256
NT = N // 128         # 16 n-tiles
KT = C // 128         # 8 contraction tiles

_nc_cache = {}


def _build_program(ln_trivial: bool):
    import concourse.bass as bass
    import concourse.mybir as mybir
    import concourse.tile as tile

    fr = mybir.dt.float32r
    f32 = mybir.dt.float32
    bf = mybir.dt.bfloat16
    AX = mybir.AxisListType
    OP = mybir.AluOpType
    ACTF = mybir.ActivationFunctionType

    nc = bass.Bass()
    xt = nc.declare_dram_parameter("xt", [C, N], bf, isOutput=False)
    wq = nc.declare_dram_parameter("wq", [C, DPC], bf, isOutput=False)
    wk = nc.declare_dram_parameter("wk", [C, DPC], bf, isOutput=False)
    wv = nc.declare_dram_parameter("wv", [C, DPC], bf, isOutput=False)
    wp = nc.declare_dram_parameter("wp", [DPC, C], bf, isOutput=False)
    ident_in = nc.declare_dram_parameter("ident", [128, 128], bf, isOutput=False)
    ones64 = nc.declare_dram_parameter("ones64", [1, D], fr, isOutput=False)
    vones = nc.declare_dram_parameter("vones", [128, D], bf, isOutput=False)
    if not ln_trivial:
        gqb = nc.declare_dram_parameter("gqb", [128, 4, D], f32, isOutput=False)
        bqb = nc.declare_dram_parameter("bqb", [128, 4, D], f32, isOutput=False)
        gkb = nc.declare_dram_parameter("gkb", [128, 4, D], f32, isOutput=False)
        bkb = nc.declare_dram_parameter("bkb", [128, 4, D], f32, isOutput=False)
    out = nc.declare_dram_parameter("out", [N, C], f32, isOutput=True)

    with tile.TileContext(nc) as tc:
        with tc.tile_pool(name="const", bufs=1) as cpool, \
             tc.tile_pool(name="persist", bufs=1) as bpool:

            ident = cpool.tile([128, 128], bf)
            nc.scalar.dma_start(ident[:], ident_in[:])
            ones_t = cpool.tile([1, D], fr)
            nc.scalar.dma_start(ones_t[:], ones64[:])
            eps_t = cpool.tile([128, 1], f32)
            nc.vector.memset(eps_t[:], EPS)
            eps64_t = cpool.tile([128, 1], f32)
            nc.vector.memset(eps64_t[:], D * EPS)

            # ---- persistent tensors (live into attention/proj) --------
            wp_s = bpool.tile([128, 2, C], bf, name="wp_s")
            v5 = bpool.tile([128, NT * HPC, 65], bf, name="v5")
            nc.scalar.dma_start(v5[:, :, 64:65], vones.rearrange("p (d o) -> p d o", o=1))
            qT = [bpool.tile([128, N], bf, name=f"qT{p}") for p in range(2)]
            kT = [bpool.tile([128, N], bf, name=f"kT{p}") for p in range(2)]
            outT = [bpool.tile([128, N], bf, name=f"outT{p}") for p in range(2)]

            with tc.tile_pool(name="ph13", bufs=1) as wpool, \
                 tc.tile_pool(name="qkv_ps", bufs=2, space="PSUM") as qps, \
                 tc.tile_pool(name="sq_pool", bufs=2) as sqpool, \
                 tc.tile_pool(name="stat_tmp", bufs=2) as stp, \
                 tc.tile_pool(name="tp_ps", bufs=2, space="PSUM") as tps:
                # ---- load inputs/weights for phases 1-3 ---------------
                wq_s = wpool.tile([128, KT, DPC], bf, name="wq_s")
                nc.scalar.dma_start(wq_s[:], wq.rearrange("(k p) d -> p k d", p=128))
                wk_s = wpool.tile([128, KT, DPC], bf, name="wk_s")
                nc.scalar.dma_start(wk_s[:], wk.rearrange("(k p) d -> p k d", p=128))
                wv_s = wpool.tile([128, KT, DPC], bf, name="wv_s")
                nc.scalar.dma_start(wv_s[:], wv.rearrange("(k p) d -> p k d", p=128))
                xt_s = wpool.tile([128, KT, N], bf, name="xt_s")
                xt_r = xt.rearrange("(k p) n -> p k n", p=128)
                # per-k-tile DMAs so the qkv accumulation (kc-ordered) can
                # start as soon as k-tile 0 lands instead of after all 4MB
                for kc in range(KT):
                    eng = nc.sync if kc % 2 == 0 else nc.scalar
                    eng.dma_start(xt_s[:, kc:kc + 1], xt_r[:, kc:kc + 1])
                # warm the PE clock right before qkv: gate on the xt tile so
                # the burst doesn't fire early and decay while DMAs stream
                wut = qps.tile([128, DPC], f32, tag="pq")
                for i in range(10):
                    nc.tensor.matmul(wut[:], xt_s[:, 0, 0:128],
                                     xt_s[:, 0, 0:DPC], start=True, stop=True)
                if not ln_trivial:
                    gq_s = wpool.tile([128, 4, D], f32, name="gq_s")
                    nc.sync.dma_start(gq_s[:], gqb[:])
                    bq_s = wpool.tile([128, 4, D], f32, name="bq_s")
                    nc.sync.dma_start(bq_s[:], bqb[:])
                    gk_s = wpool.tile([128, 4, D], f32, name="gk_s")
                    nc.sync.dma_start(gk_s[:], gkb[:])
                    bk_s = wpool.tile([128, 4, D], f32, name="bk_s")
                    nc.sync.dma_start(bk_s[:], bkb[:])

                # ---- phases 1-3, half-batched: qkv matmuls + LN stats,
                # stats finalize, apply + transpose. Processing 8-tile halves
                # lets the DVE/ACT LN work of half 0 overlap the PE qkv
                # matmuls of half 1.
                q_nat = wpool.tile([128, NT, 4, D], bf, name="q_nat")
                k_nat = wpool.tile([128, NT, 4, D], bf, name="k_nat")
                q_app = wpool.tile([128, NT, 4, D], bf, name="q_app")
                k_app = wpool.tile([128, NT, 4, D], bf, name="k_app")

                s1q = wpool.tile([128, NT, 4], f32, name="s1q")
                s2q = wpool.tile([128, NT, 4], f32, name="s2q")
                s1k = wpool.tile([128, NT, 4], f32, name="s1k")
                s2k = wpool.tile([128, NT, 4], f32, name="s2k")
                mu_q = bpool.tile([128, NT, 4], f32, name="mu_q")
                rs_q = bpool.tile([128, NT, 4], f32, name="rs_q")
                mu_k = bpool.tile([128, NT, 4], f32, name="mu_k")
                rs_k = bpool.tile([128, NT, 4], f32, name="rs_k")

                for half in range(2):
                    h8 = slice(half * 8, half * 8 + 8)
                    for t in range(half * 8, half * 8 + 8):
                        ts_ = slice(t * 128, (t + 1) * 128)
                        pq = qps.tile([128, DPC], f32, tag="pq")
                        pk = qps.tile([128, DPC], f32, tag="pk")
                        pv = qps.tile([128, DPC], f32, tag="pv")
                        for kc in range(KT):
                            st = kc == 0
                            sp = kc == KT - 1
                            lhs = xt_s[:, kc, ts_]
                            nc.tensor.matmul(pk[:], lhs, wk_s[:, kc, :],
                                             start=st, stop=sp)
                            nc.tensor.matmul(pq[:], lhs, wq_s[:, kc, :],
                                             start=st, stop=sp)
                            nc.tensor.matmul(pv[:], lhs, wv_s[:, kc, :],
                                             start=st, stop=sp)
                        # stats straight from PSUM; squares on the (idle) ACT
                        for (pp, s1, s2, natd) in ((pk, s1k, s2k, k_nat),
                                                   (pq, s1q, s2q, q_nat)):
                            pg = pp[:].rearrange("p (g d) -> p g d", g=4)
                            nc.vector.tensor_reduce(s1[:, t], pg, AX.X, OP.add)
                            sq = sqpool.tile([128, 4, D], f32, tag="sq")
                            nc.scalar.square(sq[:], pg)
                            nc.vector.tensor_reduce(s2[:, t], sq[:], AX.X, OP.add)
                            nc.scalar.copy(natd[:, t], pg)
                        nc.scalar.copy(v5[:, t * HPC:(t + 1) * HPC, 0:64],
                                       pv[:].rearrange("p (g d) -> p g d", g=4))

                    # stats finalize for this half
                    # mu = s1/64 ; var = s2/64 - mu^2 ; rstd = 1/sqrt(var+eps)
                    # Trivial-LN k path: rs_k holds SCALE*rstd; it is applied
                    # multiplicatively to k_nat below (the LN mean
                    # subtraction cancels against zero-mean q-hat in the
                    # scores), giving a uniform exp scale of 1.0.
                    for (s1, s2, mu, rs, kfold) in (
                            (s1k, s2k, mu_k, rs_k, ln_trivial),
                            (s1q, s2q, mu_q, rs_q, False)):
                        nc.vector.tensor_scalar(mu[:, h8], s1[:, h8], 1.0 / D,
                                                None, OP.mult)
                        u = stp.tile([128, 8, 4], f32, tag="u")
                        nc.vector.scalar_tensor_tensor(u[:], s1[:, h8], 1.0 / D,
                                                       s1[:, h8], OP.mult, OP.mult)
                        u2 = stp.tile([128, 8, 4], f32, tag="u2")
                        nc.vector.scalar_tensor_tensor(u2[:], u[:], -1.0,
                                                       s2[:, h8], OP.mult, OP.add)
                        if kfold:
                            # rs = 1/sqrt(64*var + 64*eps) = SCALE/sqrt(var+eps)
                            nc.scalar.activation(u[:], u2[:], ACTF.Sqrt,
                                                 bias=eps64_t[:], scale=1.0)
                        else:
                            nc.scalar.activation(u[:], u2[:], ACTF.Sqrt,
                                                 bias=eps_t[:], scale=1.0 / D)
                        nc.vector.reciprocal(rs[:, h8], u[:])

                    # apply for this half (overlaps next half's qkv on PE).
                    # Trivial path: q gets (x-mu)*rs; k gets x*rs_k with
                    # rs_k = SCALE*rstd (no mean subtraction needed).
                    for t in range(half * 8, half * 8 + 8):
                        for g in range(4):
                            nc.vector.tensor_scalar(
                                q_app[:, t, g], q_nat[:, t, g],
                                mu_q[:, t, g:g + 1], rs_q[:, t, g:g + 1],
                                OP.subtract, OP.mult)
                        if ln_trivial:
                            for g in range(4):
                                nc.vector.tensor_scalar(
                                    k_app[:, t, g], k_nat[:, t, g],
                                    rs_k[:, t, g:g + 1], None, OP.mult)
                        else:
                            for g in range(4):
                                nc.vector.tensor_scalar(
                                    k_app[:, t, g], k_nat[:, t, g],
                                    mu_k[:, t, g:g + 1], rs_k[:, t, g:g + 1],
                                    OP.subtract, OP.mult)
                            nc.vector.tensor_mul(q_app[:, t], q_app[:, t], gq_s[:])
                            nc.vector.tensor_add(q_app[:, t], q_app[:, t], bq_s[:])
                            nc.vector.tensor_mul(k_app[:, t], k_app[:, t], gk_s[:])
                            nc.vector.tensor_add(k_app[:, t], k_app[:, t], bk_s[:])

                    # transposes for this half on the PE (DMA-xbar transpose
                    # has unreliable completion ordering in this stack)
                    for (nat, dstT) in ((k_app, kT), (q_app, qT)):
                        for s in range(2):
                            for tq in range(2 * half, 2 * half + 2):
                                ptp = tps.tile([128, 4, 128], bf, tag="ptp")
                                for i in range(4):
                                    t = 4 * tq + i
                                    nc.tensor.transpose(
                                        ptp[:, i],
                                        nat[:, t, 2 * s:2 * s + 2, :],
                                        ident[:])
                                if tq % 2 == 0:
                                    nc.vector.tensor_copy(
                                        dstT[s][:, tq * 512:(tq + 1) * 512], ptp[:])
                                else:
                                    nc.scalar.copy(
                                        dstT[s][:, tq * 512:(tq + 1) * 512], ptp[:])

                if not ln_trivial:
                    # center k over sequence (softmax-invariant, kept only
                    # for the general gamma/beta path)
                    with tc.tile_pool(name="ctr", bufs=1) as ctr:
                        for p in range(2):
                            rsum = ctr.tile([128, 1], f32, tag="rsum")
                            nc.vector.tensor_reduce(rsum[:], kT[p][:], AX.X, OP.add)
                            mean = ctr.tile([128, 1], f32, tag="mean")
                            nc.vector.tensor_scalar(mean[:], rsum[:], 1.0 / N,
                                                    None, OP.mult)
                            nc.vector.tensor_scalar(kT[p][:], kT[p][:], mean[:],
                                                    None, OP.subtract)

            # ---- phase 5+6: attention per head-pair, with the per-q
            # normalize + projection + output DMA interleaved under the
            # NEXT q-chunk's (ACT-bound) attention sweeps.
            # Heads (2p, 2p+1) live on PE row strips 0-63 / 64-127 of
            # kT[p]/qT[p], so their K=64 score matmuls run CONCURRENTLY
            # (row tiling). One [128,2,512] score tile per (pair, q, mt)
            # feeds a single [128,1024] EXP - ACT does only EXP here.
            esc = 1.0 if ln_trivial else SCALE
            nc.sync.dma_start(wp_s[:], wp.rearrange("(k p) n -> p k n", p=128))
            with tc.tile_pool(name="exp_pool", bufs=3) as epool, \
                 tc.tile_pool(name="nrm_pool", bufs=1) as npool, \
                 tc.tile_pool(name="fin", bufs=4) as fpool, \
                 tc.tile_pool(name="att_ps", bufs=1, space="PSUM") as aps:
                den_all = npool.tile([1, 4, HPC, 512], f32, name="den_all")
                denr = npool.tile([1, 4, HPC, 512], fr, name="denr")
                # warm-up burst before the head loop
                wps = aps.tile([128, 2, 512], f32, tag="ps", bufs=2)
                for i in range(10):
                    nc.tensor.matmul(wps[:, i % 2], kT[0][:, 0:128],
                                     kT[0][:, 0:512], start=True, stop=True)

                def attention_sweep(hA, hB, q, inject=None):
                    p = hA // 2
                    ns = slice(q * 512, (q + 1) * 512)
                    U = {}
                    for (slot, h) in ((0, hA), (1, hB)):
                        U[slot] = aps.tile([65, 512], f32, tag=f"U{slot}",
                                           bufs=1, name=f"U{h}_{q}")
                    exs = {}

                    def emit_attnv(mt, flush=False):
                        exv = exs.pop(mt)
                        for (slot, h) in ((0, hA), (1, hB)):
                            nc.tensor.matmul(
                                U[slot][:],
                                v5[:, mt * HPC + h, :],
                                exv[:, slot, :],
                                start=(mt == 0), stop=(mt == NT - 1))
                        if flush:
                            for (slot, h) in ((0, hA), (1, hB)):
                                off = 64 * (h % 2)
                                nc.vector.tensor_copy(
                                    outT[p][off:off + 64, ns],
                                    U[slot][0:64, :])
                                nc.vector.tensor_copy(
                                    den_all[:, q, h], U[slot][64:65, :])

                    for mt in range(NT):
                        ms = slice(mt * 128, (mt + 1) * 128)
                        ps = aps.tile([128, 2, 512], f32, tag="ps", bufs=2)
                        for (slot, h) in ((0, hA), (1, hB)):
                            off = 64 * (h % 2)
                            nc.tensor.matmul(ps[:, slot],
                                             kT[p][off:off + 64, ms],
                                             qT[p][off:off + 64, ns],
                                             start=True, stop=True)
                        ex = epool.tile([128, 2, 512], bf, tag="ex", bufs=3)
                        nc.scalar.activation(
                            ex[:].rearrange("p a b -> p (a b)"),
                            ps[:].rearrange("p a b -> p (a b)"),
                            ACTF.Exp, scale=esc)
                        exs[mt] = ex
                        if mt >= 1:
                            emit_attnv(mt - 1)
                        if inject and mt in inject:
                            inject[mt]()
                    emit_attnv(NT - 1, flush=True)

                def start_recip(q):
                    # reciprocal of this q-chunk's denominators: DMA-reshape
                    # to 128 partitions so the iterative divide is parallel;
                    # runs on sync/DVE only, hidden under the next q's sweeps
                    den128 = npool.tile([128, 16], f32, tag="den128", bufs=2)
                    nc.sync.dma_start(
                        den128[:], den_all[:, q].rearrange("o h f -> o (h f)"))
                    der128 = npool.tile([128, 16], fr, tag="der128", bufs=2)
                    with nc.allow_low_precision("softmax recip"):
                        nc.vector.reciprocal(der128[:], den128[:])
                    nc.sync.dma_start(
                        denr[:, q].rearrange("o h f -> o (h f)"), der128[:])

                def make_tail_chunks(q):
                    # the PE-side tail of q-chunk q, split into small pieces
                    # that get injected into the next q's sweeps so the
                    # in-order PE queue never stalls ACT
                    ns = slice(q * 512, (q + 1) * 512)
                    chunks = []

                    def norm_piece(hp):
                        def run():
                            rbp = aps.tile([128, 2, 512], f32, tag="po", bufs=1)
                            for j in range(2):
                                h = 2 * hp + j
                                nc.tensor.matmul(rbp[0:64, j, :],
                                                 ones_t[:], denr[:, q, h],
                                                 start=True, stop=True)
                            for j in range(2):
                                h = 2 * hp + j
                                p = h // 2
                                off = 64 * (h % 2)
                                sl = outT[p][off:off + 64, ns]
                                nc.vector.tensor_mul(sl, sl, rbp[0:64, j, :])
                        return run

                    def proj_piece(t):
                        def run():
                            ts_ = slice(t * 128, (t + 1) * 128)
                            po = aps.tile([128, 2, 512], f32, tag="po", bufs=1)
                            for p in range(2):
                                for j in range(2):
                                    nc.tensor.matmul(
                                        po[:, j], outT[p][:, ts_],
                                        wp_s[:, p, j * 512:(j + 1) * 512],
                                        start=(p == 0), stop=(p == 1))
                            fin = fpool.tile([128, 1024], f32, tag="fin")
                            nc.vector.tensor_copy(
                                fin[:], po[:].rearrange("p a b -> p (a b)"))
                            nc.sync.dma_start(out[ts_, :], fin[:])
                        return run

                    chunks.append(norm_piece(0))
                    chunks.append(norm_piece(1))
                    for t in range(4 * q, 4 * q + 4):
                        chunks.append(proj_piece(t))
                    return chunks

                for q in range(4):
                    if q >= 1:
                        tail = make_tail_chunks(q - 1)
                        inj0 = {5: tail[0], 8: tail[1], 11: tail[2],
                                14: tail[3]}
                        inj1 = {3: tail[4], 8: tail[5]}
                    else:
                        inj0 = inj1 = None
                    attention_sweep(0, 1, q, inject=inj0)
                    attention_sweep(2, 3, q, inject=inj1)
                    start_recip(q)
                for piece in make_tail_chunks(3):
                    piece()

    return nc


def _get_program(ln_trivial: bool):
    key = ln_trivial
    if key not in _nc_cache:
        _install_patch()
        _nc_cache[key] = _build_program(ln_trivial)
    return _nc_cache[key]


def _bf16():
    import ml_dtypes
    return ml_dtypes.bfloat16


def _prep_core_inputs(c, x, qkv_w, q_norm_w, q_norm_b, k_norm_w, k_norm_b,
                      proj_w, ln_trivial):
    b = c // 4
    g = c % 4
    rows = slice(g * DPC, (g + 1) * DPC)
    b16 = _bf16()
    xt = np.ascontiguousarray(x[b].T).astype(b16)           # [C, N]
    wq = np.ascontiguousarray(qkv_w[rows, :].T).astype(b16)  # [C, DPC]
    wk = np.ascontiguousarray(qkv_w[C:2 * C, :][rows, :].T).astype(b16)
    wv = np.ascontiguousarray(qkv_w[2 * C:3 * C, :][rows, :].T).astype(b16)
    wp = np.ascontiguousarray(proj_w[:, rows].T).astype(b16)  # [DPC, C]
    m = {"xt": xt, "wq": wq, "wk": wk, "wv": wv, "wp": wp,
         "ident": np.eye(128, dtype=_bf16()),
         "ones64": np.ones((1, D), dtype=np.float32),
         "vones": np.ones((128, D), dtype=_bf16())}
    if not ln_trivial:
        for nm, arr in (("gqb", q_norm_w), ("bqb", q_norm_b),
                        ("gkb", k_norm_w), ("bkb", k_norm_b)):
            t = np.broadcast_to(arr.astype(np.float32), (128, 4, D))
            m[nm] = np.ascontiguousarray(t)
    return m


def kernel(x, qkv_w, q_norm_w, q_norm_b, k_norm_w, k_norm_b, proj_w, proj_b,
           _trace=False):
    from concourse.bass_utils import run_bass_kernel_spmd

    x = np.asarray(x, dtype=np.float32)
    qkv_w = np.asarray(qkv_w, dtype=np.float32)
    q_norm_w = np.asarray(q_norm_w, dtype=np.float32)
    q_norm_b = np.asarray(q_norm_b, dtype=np.float32)
    k_norm_w = np.asarray(k_norm_w, dtype=np.float32)
    k_norm_b = np.asarray(k_norm_b, dtype=np.float32)
    proj_w = np.asarray(proj_w, dtype=np.float32)
    proj_b = np.asarray(proj_b, dtype=np.float32)

    ln_trivial = (np.all(q_norm_w == 1.0) and np.all(q_norm_b == 0.0)
                  and np.all(k_norm_w == 1.0) and np.all(k_norm_b == 0.0))

    nc = _get_program(ln_trivial)
    in_maps = [
        _prep_core_inputs(c, x, qkv_w, q_norm_w, q_norm_b, k_norm_w,
                          k_norm_b, proj_w, ln_trivial)
        for c in range(NCORES)
    ]
    res = run_bass_kernel_spmd(nc, in_maps, list(range(NCORES)),
                               trace=_trace)
    outs = [res.results[c]["out"] for c in range(NCORES)]
    full = np.empty((B, N, C), dtype=np.float32)
    for b in range(B):
        acc = outs[4 * b].astype(np.float32)
        for g in range(1, 4):
            acc = acc + outs[4 * b + g]
        full[b] = acc + proj_b[None, :]
    if _trace:
        return full, res
    return full


# revision 18
# speedup vs baseline: 1.2576x; 1.0430x over previous
"""Trainium2 Bass kernel for nn_Attention_32091995636193.

Dense transformer attention block:
  qkv = x @ qkv_w.T ; per-head LN(q), LN(k) over head_dim ; k centered over
  seq ; softmax(q*scale @ k^T) @ v ; out @ proj_w.T + proj_b.

Sharding over 8 NeuronCores: data parallel on batch (B=2) x tensor parallel
on heads (16 heads -> 4 per core). Core c handles batch c//4, heads
4*(c%4) .. 4*(c%4)+3. Each core computes its partial projection output
[N, C]; the host sums the 4 partials per batch and adds proj_b.

Per-core device program (bf16 data, fp32 psum):
  1. qT/kT/v from xT and weight slices (natural [n,d] layout)
  2. LayerNorm stats from PSUM, finalize, apply. Trivial-gamma path folds
     SCALE*rstd_k into a k-scale pass (LN mean of k cancels against
     zero-mean q-hat inside the softmax; see notes below), so exp runs
     with a uniform scale of 1.0.
  3. q/k -> [d, n] layout via DMA-xbar transposes (no PE time)
  4. Attention per head-PAIR: the two heads of a pair live on PE row
     strips 0-63 / 64-127, so their K=64 score matmuls execute
     concurrently (row tiling). Per (pair, n-quarter, mt): one
     [128,2,512] score tile -> one [128,1024] EXP on ACT -> two [65,512]
     attnv matmuls accumulating U (row 64 = ones = softmax denominator).
     ACT does nothing but EXP in this phase - it is the wall.
  5. deferred normalize: reciprocal of denominators + ones-matmul
     broadcast + DVE multiply
  6. projection partial, streamed out per 128-row tile
"""

import os
import sys

for _p in ("/opt/trn_rl_repo",):
    if _p not in sys.path and os.path.isdir(_p):
        sys.path.append(_p)

import numpy as np

# ---------------------------------------------------------------------------
# BIR legalizer: the pinned walrus build supports at most ONE sync wait per
# instruction, but Tile's scheduler attaches several. Split extra waits onto
# NoOp instructions inserted immediately before (same engine => same NX
# order => identical semantics).
# ---------------------------------------------------------------------------
import orjson


def _legalize_bir_json_bytes(raw: bytes) -> bytes:
    j = orjson.loads(raw)
    counter = 0
    for f in j.get("functions", []):
        for blk in f.get("blocks", []):
            insts = blk.get("instructions")
            if not insts:
                continue
            out = []
            for ins in insts:
                si = ins.get("sync_info")
                waits = si.get("on_wait") if si else None
                if waits and len(waits) > 1:
                    engine = ins.get("engine")
                    for w in waits[:-1]:
                        counter += 1
                        nop = {
                            "name": f"WSPLIT-{counter}",
                            "opcode": "NoOp",
                            "engine": engine,
                            "ins": [],
                            "outs": [],
                            "sync_info": {"on_wait": [w], "on_update": []},
                        }
                        if "debug" in ins:
                            nop["debug"] = ins["debug"]
                        out.append(nop)
                    si["on_wait"] = [waits[-1]]
                out.append(ins)
            blk["instructions"] = out
    return orjson.dumps(j)


_PATCHED = False


def _install_patch():
    global _PATCHED
    if _PATCHED:
        return
    import concourse.bass as bass

    orig = bass.Bass.to_json_bytes

    def patched(self):
        return _legalize_bir_json_bytes(orig(self))

    bass.Bass.to_json_bytes = patched
    _PATCHED = True


# ---------------------------------------------------------------------------
# Problem constants (hardcoded per the harness contract)
# ---------------------------------------------------------------------------
B = 2
N = 2048
C = 1024
H = 16
D = 64
SCALE = D ** -0.5
EPS = 1e-5
NCORES = 8
HPC = H // 4          # heads per core = 4
DPC = HPC * D         # channels per core = 256
NT = N // 128         # 16 n-tiles
KT = C // 128         # 8 contraction tiles

_nc_cache = {}


def _build_program(ln_trivial: bool):
    import concourse.bass as bass
    import concourse.mybir as mybir
    import concourse.tile as tile

    fr = mybir.dt.float32r
    f32 = mybir.dt.float32
    bf = mybir.dt.bfloat16
    AX = mybir.AxisListType
    OP = mybir.AluOpType
    ACTF = mybir.ActivationFunctionType

    nc = bass.Bass()
    xt = nc.declare_dram_parameter("xt", [C, N], bf, isOutput=False)
    wq = nc.declare_dram_parameter("wq", [C, DPC], bf, isOutput=False)
    wk = nc.declare_dram_parameter("wk", [C, DPC], bf, isOutput=False)
    wv = nc.declare_dram_parameter("wv", [C, DPC], bf, isOutput=False)
    wp = nc.declare_dram_parameter("wp", [DPC, C], bf, isOutput=False)
    ident_in = nc.declare_dram_parameter("ident", [128, 128], bf, isOutput=False)
    ones64 = nc.declare_dram_parameter("ones64", [1, D], fr, isOutput=False)
    vones = nc.declare_dram_parameter("vones", [128, D], bf, isOutput=False)
    if not ln_trivial:
        gqb = nc.declare_dram_parameter("gqb", [128, 4, D], f32, isOutput=False)
        bqb = nc.declare_dram_parameter("bqb", [128, 4, D], f32, isOutput=False)
        gkb = nc.declare_dram_parameter("gkb", [128, 4, D], f32, isOutput=False)
        bkb = nc.declare_dram_parameter("bkb", [128, 4, D], f32, isOutput=False)
    out = nc.declare_dram_parameter("out", [N, C], f32, isOutput=True)

    with tile.TileContext(nc) as tc:
        with tc.tile_pool(name="const", bufs=1) as cpool, \
             tc.tile_pool(name="persist", bufs=1) as bpool:

            ident = cpool.tile([128, 128], bf)
            ones_t = cpool.tile([1, D], fr)
            eps_t = cpool.tile([128, 1], f32)
            nc.vector.memset(eps_t[:], EPS)
            eps64_t = cpool.tile([128, 1], f32)
            nc.vector.memset(eps64_t[:], D * EPS)

            # ---- persistent tensors (live into attention/proj) --------
            wp_s = bpool.tile([128, 2, C], bf, name="wp_s")
            v5 = bpool.tile([128, NT * HPC, 65], bf, name="v5")
            qT = [bpool.tile([128, N], bf, name=f"qT{p}") for p in range(2)]
            kT = [bpool.tile([128, N], bf, name=f"kT{p}") for p in range(2)]
            outT = [bpool.tile([128, N], bf, name=f"outT{p}") for p in range(2)]

            with tc.tile_pool(name="ph13", bufs=1) as wpool, \
                 tc.tile_pool(name="qkv_ps", bufs=2, space="PSUM") as qps, \
                 tc.tile_pool(name="sq_pool", bufs=2) as sqpool, \
                 tc.tile_pool(name="stat_tmp", bufs=2) as stp, \
                 tc.tile_pool(name="tp_ps", bufs=2, space="PSUM") as tps:
                # ---- load inputs/weights for phases 1-3 ---------------
                wq_s = wpool.tile([128, KT, DPC], bf, name="wq_s")
                nc.scalar.dma_start(wq_s[:], wq.rearrange("(k p) d -> p k d", p=128))
                wk_s = wpool.tile([128, KT, DPC], bf, name="wk_s")
                nc.scalar.dma_start(wk_s[:], wk.rearrange("(k p) d -> p k d", p=128))
                wv_s = wpool.tile([128, KT, DPC], bf, name="wv_s")
                nc.scalar.dma_start(wv_s[:], wv.rearrange("(k p) d -> p k d", p=128))
                # ident/ones/v5-ones queued on scalar AFTER the weights
                # (same queue as the stable config, weights first)
                nc.scalar.dma_start(ident[:], ident_in[:])
                nc.scalar.dma_start(ones_t[:], ones64[:])
                nc.scalar.dma_start(v5[:, :, 64:65],
                                    vones.rearrange("p (d o) -> p d o", o=1))
                xt_s = wpool.tile([128, KT, N], bf, name="xt_s")
                xt_r = xt.rearrange("(k p) n -> p k n", p=128)
                # per-k-tile DMAs so the qkv accumulation (kc-ordered) can
                # start as soon as k-tile 0 lands instead of after all 4MB
                for kc in range(KT):
                    eng = nc.sync if kc % 2 == 0 else nc.scalar
                    eng.dma_start(xt_s[:, kc:kc + 1], xt_r[:, kc:kc + 1])
                # warm the PE clock right before qkv: gate on the xt tile so
                # the burst doesn't fire early and decay while DMAs stream
                wut = qps.tile([128, DPC], f32, tag="pq")
                for i in range(10):
                    nc.tensor.matmul(wut[:], xt_s[:, 0, 0:128],
                                     xt_s[:, 0, 0:DPC], start=True, stop=True)
                if not ln_trivial:
                    gq_s = wpool.tile([128, 4, D], f32, name="gq_s")
                    nc.sync.dma_start(gq_s[:], gqb[:])
                    bq_s = wpool.tile([128, 4, D], f32, name="bq_s")
                    nc.sync.dma_start(bq_s[:], bqb[:])
                    gk_s = wpool.tile([128, 4, D], f32, name="gk_s")
                    nc.sync.dma_start(gk_s[:], gkb[:])
                    bk_s = wpool.tile([128, 4, D], f32, name="bk_s")
                    nc.sync.dma_start(bk_s[:], bkb[:])

                # ---- phases 1-3, half-batched: qkv matmuls + LN stats,
                # stats finalize, apply + transpose. Processing 8-tile halves
                # lets the DVE/ACT LN work of half 0 overlap the PE qkv
                # matmuls of half 1.
                q_nat = wpool.tile([128, NT, 4, D], bf, name="q_nat")
                k_nat = wpool.tile([128, NT, 4, D], bf, name="k_nat")
                q_app = wpool.tile([128, NT, 4, D], bf, name="q_app")
                k_app = wpool.tile([128, NT, 4, D], bf, name="k_app")

                s1q = wpool.tile([128, NT, 4], f32, name="s1q")
                s2q = wpool.tile([128, NT, 4], f32, name="s2q")
                s1k = wpool.tile([128, NT, 4], f32, name="s1k")
                s2k = wpool.tile([128, NT, 4], f32, name="s2k")
                mu_q = bpool.tile([128, NT, 4], f32, name="mu_q")
                rs_q = bpool.tile([128, NT, 4], f32, name="rs_q")
                mu_k = bpool.tile([128, NT, 4], f32, name="mu_k")
                rs_k = bpool.tile([128, NT, 4], f32, name="rs_k")

                for half in range(2):
                    h8 = slice(half * 8, half * 8 + 8)
                    for t in range(half * 8, half * 8 + 8):
                        ts_ = slice(t * 128, (t + 1) * 128)
                        pq = qps.tile([128, DPC], f32, tag="pq")
                        pk = qps.tile([128, DPC], f32, tag="pk")
                        pv = qps.tile([128, DPC], f32, tag="pv")
                        for kc in range(KT):
                            st = kc == 0
                            sp = kc == KT - 1
                            lhs = xt_s[:, kc, ts_]
                            nc.tensor.matmul(pk[:], lhs, wk_s[:, kc, :],
                                             start=st, stop=sp)
                            nc.tensor.matmul(pq[:], lhs, wq_s[:, kc, :],
                                             start=st, stop=sp)
                            nc.tensor.matmul(pv[:], lhs, wv_s[:, kc, :],
                                             start=st, stop=sp)
                        # stats straight from PSUM; squares on the (idle) ACT
                        for (pp, s1, s2, natd) in ((pk, s1k, s2k, k_nat),
                                                   (pq, s1q, s2q, q_nat)):
                            pg = pp[:].rearrange("p (g d) -> p g d", g=4)
                            nc.vector.tensor_reduce(s1[:, t], pg, AX.X, OP.add)
                            sq = sqpool.tile([128, 4, D], f32, tag="sq")
                            nc.scalar.square(sq[:], pg)
                            nc.vector.tensor_reduce(s2[:, t], sq[:], AX.X, OP.add)
                            nc.scalar.copy(natd[:, t], pg)
                        nc.scalar.copy(v5[:, t * HPC:(t + 1) * HPC, 0:64],
                                       pv[:].rearrange("p (g d) -> p g d", g=4))

                    # stats finalize for this half
                    # mu = s1/64 ; var = s2/64 - mu^2 ; rstd = 1/sqrt(var+eps)
                    # Trivial-LN k path: rs_k holds SCALE*rstd; it is applied
                    # multiplicatively to k_nat below (the LN mean
                    # subtraction cancels against zero-mean q-hat in the
                    # scores), giving a uniform exp scale of 1.0.
                    for (s1, s2, mu, rs, kfold) in (
                            (s1k, s2k, mu_k, rs_k, ln_trivial),
                            (s1q, s2q, mu_q, rs_q, False)):
                        nc.vector.tensor_scalar(mu[:, h8], s1[:, h8], 1.0 / D,
                                                None, OP.mult)
                        u = stp.tile([128, 8, 4], f32, tag="u")
                        nc.vector.scalar_tensor_tensor(u[:], s1[:, h8], 1.0 / D,
                                                       s1[:, h8], OP.mult, OP.mult)
                        u2 = stp.tile([128, 8, 4], f32, tag="u2")
                        nc.vector.scalar_tensor_tensor(u2[:], u[:], -1.0,
                                                       s2[:, h8], OP.mult, OP.add)
                        if kfold:
                            # rs = 1/sqrt(64*var + 64*eps) = SCALE/sqrt(var+eps)
                            nc.scalar.activation(u[:], u2[:], ACTF.Sqrt,
                                                 bias=eps64_t[:], scale=1.0)
                        else:
                            nc.scalar.activation(u[:], u2[:], ACTF.Sqrt,
                                                 bias=eps_t[:], scale=1.0 / D)
                        nc.vector.reciprocal(rs[:, h8], u[:])

                    # apply for this half (overlaps next half's qkv on PE).
                    # Trivial path: q gets (x-mu)*rs; k gets x*rs_k with
                    # rs_k = SCALE*rstd (no mean subtraction needed).
                    for t in range(half * 8, half * 8 + 8):
                        for g in range(4):
                            nc.vector.tensor_scalar(
                                q_app[:, t, g], q_nat[:, t, g],
                                mu_q[:, t, g:g + 1], rs_q[:, t, g:g + 1],
                                OP.subtract, OP.mult)
                        if ln_trivial:
                            for g in range(4):
                                nc.vector.tensor_scalar(
                                    k_app[:, t, g], k_nat[:, t, g],
                                    rs_k[:, t, g:g + 1], None, OP.mult)
                        else:
                            for g in range(4):
                                nc.vector.tensor_scalar(
                                    k_app[:, t, g], k_nat[:, t, g],
                                    mu_k[:, t, g:g + 1], rs_k[:, t, g:g + 1],
                                    OP.subtract, OP.mult)
                            nc.vector.tensor_mul(q_app[:, t], q_app[:, t], gq_s[:])
                            nc.vector.tensor_add(q_app[:, t], q_app[:, t], bq_s[:])
                            nc.vector.tensor_mul(k_app[:, t], k_app[:, t], gk_s[:])
                            nc.vector.tensor_add(k_app[:, t], k_app[:, t], bk_s[:])

                    # transposes for this half on the PE (DMA-xbar transpose
                    # has unreliable completion ordering in this stack)
                    for (nat, dstT) in ((k_app, kT), (q_app, qT)):
                        for s in range(2):
                            for tq in range(2 * half, 2 * half + 2):
                                ptp = tps.tile([128, 4, 128], bf, tag="ptp")
                                for i in range(4):
                                    t = 4 * tq + i
                                    nc.tensor.transpose(
                                        ptp[:, i],
                                        nat[:, t, 2 * s:2 * s + 2, :],
                                        ident[:])
                                if tq % 2 == 0:
                                    nc.vector.tensor_copy(
                                        dstT[s][:, tq * 512:(tq + 1) * 512], ptp[:])
                                else:
                                    nc.scalar.copy(
                                        dstT[s][:, tq * 512:(tq + 1) * 512], ptp[:])

                if not ln_trivial:
                    # center k over sequence (softmax-invariant, kept only
                    # for the general gamma/beta path)
                    with tc.tile_pool(name="ctr", bufs=1) as ctr:
                        for p in range(2):
                            rsum = ctr.tile([128, 1], f32, tag="rsum")
                            nc.vector.tensor_reduce(rsum[:], kT[p][:], AX.X, OP.add)
                            mean = ctr.tile([128, 1], f32, tag="mean")
                            nc.vector.tensor_scalar(mean[:], rsum[:], 1.0 / N,
                                                    None, OP.mult)
                            nc.vector.tensor_scalar(kT[p][:], kT[p][:], mean[:],
                                                    None, OP.subtract)

            # ---- phase 5+6: attention per head-pair, with the per-q
            # normalize + projection + output DMA interleaved under the
            # NEXT q-chunk's (ACT-bound) attention sweeps.
            # Heads (2p, 2p+1) live on PE row strips 0-63 / 64-127 of
            # kT[p]/qT[p], so their K=64 score matmuls run CONCURRENTLY
            # (row tiling). One [128,2,512] score tile per (pair, q, mt)
            # feeds a single [128,1024] EXP - ACT does only EXP here.
            esc = 1.0 if ln_trivial else SCALE
            nc.sync.dma_start(wp_s[:], wp.rearrange("(k p) n -> p k n", p=128))
            with tc.tile_pool(name="exp_pool", bufs=3) as epool, \
                 tc.tile_pool(name="nrm_pool", bufs=1) as npool, \
                 tc.tile_pool(name="fin", bufs=4) as fpool, \
                 tc.tile_pool(name="att_ps", bufs=1, space="PSUM") as aps:
                den_all = npool.tile([1, 4, HPC, 512], f32, name="den_all")
                denr = npool.tile([1, 4, HPC, 512], fr, name="denr")
                # warm-up burst before the head loop
                wps = aps.tile([128, 2, 512], f32, tag="ps", bufs=2)
                for i in range(10):
                    nc.tensor.matmul(wps[:, i % 2], kT[0][:, 0:128],
                                     kT[0][:, 0:512], start=True, stop=True)

                def attention_sweep(hA, hB, q, inject=None):
                    p = hA // 2
                    ns = slice(q * 512, (q + 1) * 512)
                    U = {}
                    for (slot, h) in ((0, hA), (1, hB)):
                        U[slot] = aps.tile([65, 512], f32, tag=f"U{slot}",
                                           bufs=1, name=f"U{h}_{q}")
                    exs = {}

                    def emit_attnv(mt, flush=False):
                        exv = exs.pop(mt)
                        for (slot, h) in ((0, hA), (1, hB)):
                            nc.tensor.matmul(
                                U[slot][:],
                                v5[:, mt * HPC + h, :],
                                exv[:, slot, :],
                                start=(mt == 0), stop=(mt == NT - 1))
                        if flush:
                            for (slot, h) in ((0, hA), (1, hB)):
                                off = 64 * (h % 2)
                                nc.vector.tensor_copy(
                                    outT[p][off:off + 64, ns],
                                    U[slot][0:64, :])
                                nc.vector.tensor_copy(
                                    den_all[:, q, h], U[slot][64:65, :])

                    for mt in range(NT):
                        ms = slice(mt * 128, (mt + 1) * 128)
                        ps = aps.tile([128, 2, 512], f32, tag="ps", bufs=2)
                        for (slot, h) in ((0, hA), (1, hB)):
                            off = 64 * (h % 2)
                            nc.tensor.matmul(ps[:, slot],
                                             kT[p][off:off + 64, ms],
                                             qT[p][off:off + 64, ns],
                                             start=True, stop=True)
                        ex = epool.tile([128, 2, 512], bf, tag="ex", bufs=3)
                        nc.scalar.activation(
                            ex[:].rearrange("p a b -> p (a b)"),
                            ps[:].rearrange("p a b -> p (a b)"),
                            ACTF.Exp, scale=esc)
                        exs[mt] = ex
                        if mt >= 1:
                            emit_attnv(mt - 1)
                        if inject and mt in inject:
                            inject[mt]()
                    emit_attnv(NT - 1, flush=True)

                def start_recip(q, hp):
                    # reciprocal of one head-pair's denominators: DMA-reshape
                    # to 128 partitions so the iterative divide is parallel;
                    # runs on sync/DVE only, hidden under following sweeps
                    hs = slice(2 * hp, 2 * hp + 2)
                    den128 = npool.tile([128, 8], f32, tag="den128", bufs=2)
                    nc.sync.dma_start(
                        den128[:], den_all[:, q, hs].rearrange("o h f -> o (h f)"))
                    der128 = npool.tile([128, 8], fr, tag="der128", bufs=2)
                    with nc.allow_low_precision("softmax recip"):
                        nc.vector.reciprocal(der128[:], den128[:])
                    nc.sync.dma_start(
                        denr[:, q, hs].rearrange("o h f -> o (h f)"), der128[:])

                def make_tail_chunks(q):
                    # the PE-side tail of q-chunk q, split into small pieces
                    # that get injected into the next q's sweeps so the
                    # in-order PE queue never stalls ACT
                    ns = slice(q * 512, (q + 1) * 512)
                    chunks = []

                    def norm_piece(hp):
                        def run():
                            rbp = aps.tile([128, 2, 512], f32, tag="po", bufs=1)
                            for j in range(2):
                                h = 2 * hp + j
                                nc.tensor.matmul(rbp[0:64, j, :],
                                                 ones_t[:], denr[:, q, h],
                                                 start=True, stop=True)
                            for j in range(2):
                                h = 2 * hp + j
                                p = h // 2
                                off = 64 * (h % 2)
                                sl = outT[p][off:off + 64, ns]
                                nc.vector.tensor_mul(sl, sl, rbp[0:64, j, :])
                        return run

                    state = {}

                    def proj_piece(t, j):
                        def run():
                            ts_ = slice(t * 128, (t + 1) * 128)
                            if j == 0:
                                state[t] = aps.tile([128, 2, 512], f32,
                                                    tag="po", bufs=1,
                                                    name=f"po_{t}")
                            po = state[t]
                            for p in range(2):
                                nc.tensor.matmul(
                                    po[:, j], outT[p][:, ts_],
                                    wp_s[:, p, j * 512:(j + 1) * 512],
                                    start=(p == 0), stop=(p == 1))
                            if j == 1:
                                fin = fpool.tile([128, 1024], f32, tag="fin")
                                nc.vector.tensor_copy(
                                    fin[:], po[:].rearrange("p a b -> p (a b)"))
                                nc.sync.dma_start(out[ts_, :], fin[:])
                                del state[t]
                        return run

                    chunks.append(norm_piece(0))
                    chunks.append(norm_piece(1))
                    for t in range(4 * q, 4 * q + 4):
                        chunks.append(proj_piece(t, 0))
                        chunks.append(proj_piece(t, 1))
                    return chunks

                for q in range(4):
                    if q >= 1:
                        tail = make_tail_chunks(q - 1)
                        inj0 = {3: tail[0], 5: tail[1], 7: tail[2],
                                9: tail[3], 11: tail[4], 13: tail[5]}
                        inj1 = {3: tail[6], 7: tail[7], 11: tail[8],
                                14: tail[9]}
                    else:
                        inj0 = inj1 = None
                    attention_sweep(0, 1, q, inject=inj0)
                    start_recip(q, 0)
                    attention_sweep(2, 3, q, inject=inj1)
                    start_recip(q, 1)
                # last q-chunk tail: the first norm piece can hide inside the
                # final sweep via the per-pair recip; the rest is exposed
                for piece in make_tail_chunks(3):
                    piece()

    return nc


def _get_program(ln_trivial: bool):
    key = ln_trivial
    if key not in _nc_cache:
        _install_patch()
        _nc_cache[key] = _build_program(ln_trivial)
    return _nc_cache[key]


def _bf16():
    import ml_dtypes
    return ml_dtypes.bfloat16


def _prep_core_inputs(c, x, qkv_w, q_norm_w, q_norm_b, k_norm_w, k_norm_b,
                      proj_w, ln_trivial):
    b = c // 4
    g = c % 4
    rows = slice(g * DPC, (g + 1) * DPC)
    b16 = _bf16()
    xt = np.ascontiguousarray(x[b].T).astype(b16)           # [C, N]
    wq = np.ascontiguousarray(qkv_w[rows, :].T).astype(b16)  # [C, DPC]
    wk = np.ascontiguousarray(qkv_w[C:2 * C, :][rows, :].T).astype(b16)
    wv = np.ascontiguousarray(qkv_w[2 * C:3 * C, :][rows, :].T).astype(b16)
    wp = np.ascontiguousarray(proj_w[:, rows].T).astype(b16)  # [DPC, C]
    m = {"xt": xt, "wq": wq, "wk": wk, "wv": wv, "wp": wp,
         "ident": np.eye(128, dtype=_bf16()),
         "ones64": np.ones((1, D), dtype=np.float32),
         "vones": np.ones((128, D), dtype=_bf16())}
    if not ln_trivial:
        for nm, arr in (("gqb", q_norm_w), ("bqb", q_norm_b),
                        ("gkb", k_norm_w), ("bkb", k_norm_b)):
            t = np.broadcast_to(arr.astype(np.float32), (128, 4, D))
            m[nm] = np.ascontiguousarray(t)
    return m


def kernel(x, qkv_w, q_norm_w, q_norm_b, k_norm_w, k_norm_b, proj_w, proj_b,
           _trace=False):
    from concourse.bass_utils import run_bass_kernel_spmd

    x = np.asarray(x, dtype=np.float32)
    qkv_w = np.asarray(qkv_w, dtype=np.float32)
    q_norm_w = np.asarray(q_norm_w, dtype=np.float32)
    q_norm_b = np.asarray(q_norm_b, dtype=np.float32)
    k_norm_w = np.asarray(k_norm_w, dtype=np.float32)
    k_norm_b = np.asarray(k_norm_b, dtype=np.float32)
    proj_w = np.asarray(proj_w, dtype=np.float32)
    proj_b = np.asarray(proj_b, dtype=np.float32)

    ln_trivial = (np.all(q_norm_w == 1.0) and np.all(q_norm_b == 0.0)
                  and np.all(k_norm_w == 1.0) and np.all(k_norm_b == 0.0))

    nc = _get_program(ln_trivial)
    in_maps = [
        _prep_core_inputs(c, x, qkv_w, q_norm_w, q_norm_b, k_norm_w,
                          k_norm_b, proj_w, ln_trivial)
        for c in range(NCORES)
    ]
    res = run_bass_kernel_spmd(nc, in_maps, list(range(NCORES)),
                               trace=_trace)
    outs = [res.results[c]["out"] for c in range(NCORES)]
    full = np.empty((B, N, C), dtype=np.float32)
    for b in range(B):
        acc = outs[4 * b].astype(np.float32)
        for g in range(1, 4):
            acc = acc + outs[4 * b + g]
        full[b] = acc + proj_b[None, :]
    if _trace:
        return full, res
    return full
